# revision 1
# baseline (speedup 1.0000x reference)
"""Trainium2 Bass kernel for causal multi-head attention (B=2, S=2048, E=1024, H=16).

Sharding: 8 cores = 2 batches x 4 head-groups (4 heads each).
Each core computes its batch's QKV for its 4 heads, causal attention, and a
partial output projection; host sums the 4 group partials per batch + b_out.

All big matmuls run in float32r (TF32-like, 1 cycle/row at N>=256).
"""
import sys

sys.path.insert(0, "/opt/trn_rl_repo")

from contextlib import ExitStack

import numpy as np

import concourse.bass as bass
import concourse.tile as tile
from concourse import bacc, mybir
from concourse.bass_utils import run_bass_kernel_spmd

dt = mybir.dt

B, S, E, H = 2, 2048, 1024, 16
HD = 64                     # head dim
HPC = 4                     # heads per core
NC = 8                      # cores
KE = E // 128               # 8 contraction k-tiles for projections
NT = S // 128               # 16 token tiles
NCH = S // 512              # 4 token chunks
FQK = 512                   # q+k features per core (4 heads * 64 * 2)
FV = 256                    # v features per core

# engine used for fp32 -> fp32r rounding copies of DMA'd inputs
ROUND_ENGINE = "gpsimd"


def _build_program():
    nc = bacc.Bacc("TRN2", target_bir_lowering=False, debug=False, num_devices=NC)

    xT_d = nc.dram_tensor("xT", [E, S], dt.float32, kind="ExternalInput")
    wqkT_d = nc.dram_tensor("wqkT", [E, FQK], dt.float32, kind="ExternalInput")
    wvT_d = nc.dram_tensor("wvT", [E, FV], dt.float32, kind="ExternalInput")
    bqk_d = nc.dram_tensor("bqk", [FQK], dt.float32, kind="ExternalInput")
    bv_d = nc.dram_tensor("bv", [FV], dt.float32, kind="ExternalInput")
    wo_d = nc.dram_tensor("wo", [FV, E], dt.float32, kind="ExternalInput")
    mask_d = nc.dram_tensor("trimask", [128, 128], dt.float32, kind="ExternalInput")
    y_d = nc.dram_tensor("y", [S, E], dt.float32, kind="ExternalOutput")

    with TileKernel(nc) as tk:
        tk.build(xT_d, wqkT_d, wvT_d, bqk_d, bv_d, wo_d, mask_d, y_d)
    nc.compile()
    return nc


class TileKernel:
    def __init__(self, nc):
        self.nc = nc
        self.ctx = ExitStack()
        self.tc_cm = tile.TileContext(nc)

    def __enter__(self):
        self.tc = self.tc_cm.__enter__()
        return self

    def __exit__(self, *a):
        self.ctx.close()
        return self.tc_cm.__exit__(*a)

    _round_i = 0

    def round_copy(self, out, in_):
        nc = self.nc
        nc.vector.tensor_copy(out, in_)

    def build(self, xT_d, wqkT_d, wvT_d, bqk_d, bv_d, wo_d, mask_d, y_d):
        nc, tc, ctx = self.nc, self.tc, self.ctx
        pool = lambda name, bufs, **kw: ctx.enter_context(
            tc.tile_pool(name=name, bufs=bufs, **kw)
        )

        const_p = pool("const", 1)
        xs_p = pool("xs", 1)
        xr_p = pool("xr", 2)
        qkt_p = pool("qkt", 1)
        vones_p = pool("vones", 1)
        attn_p = pool("attn", 3)
        pair_p = pool("pair", 1)
        small_p = pool("small", 1)
        y_p = pool("y", 4)
        # PSUM: ps (2 banks x 2 bufs) + po (1 bank x 1 buf x 4 tags) = 8
        ps_p = pool("ps", 2, space="PSUM")
        po_p = pool("po", 1, space="PSUM")
        p1_p = ps_p  # qkv/outproj psums share the ps slots

        # ---- weights ----

        wstage_cm = tc.tile_pool(name="wstage", bufs=1)
        wstage_p = wstage_cm.__enter__()
        wqk_st = wstage_p.tile([128, KE * FQK], dt.float32, tag="wst")
        wqk_big = const_p.tile([128, KE * FQK], dt.float32r, tag="wqk")
        for h in range(2):
            hs = slice(h * (KE // 2) * FQK, (h + 1) * (KE // 2) * FQK)
            nc.sync.dma_start(
                wqk_st[:, hs].rearrange("p (ke f) -> p ke f", f=FQK),
                wqkT_d[h * 512 : (h + 1) * 512, :].rearrange("(ke p) f -> p ke f", p=128),
            )
            self.round_copy(wqk_big[:, hs], wqk_st[:, hs])
        wqk_r = [wqk_big[:, FQK * ke : FQK * (ke + 1)] for ke in range(KE)]

        wv_st = wstage_p.tile([128, KE * FV], dt.float32, tag="wst")
        nc.sync.dma_start(
            wv_st[:].rearrange("p (ke f) -> p ke f", f=FV),
            wvT_d[:].rearrange("(ke p) f -> p ke f", p=128),
        )
        wv_big = const_p.tile([128, KE * FV], dt.float32r, tag="wv")
        self.round_copy(wv_big[:], wv_st[:])
        wv_r = [wv_big[:, FV * ke : FV * (ke + 1)] for ke in range(KE)]

        wo_st = wstage_p.tile([128, 2 * E], dt.float32, tag="wst")
        nc.sync.dma_start(
            wo_st[:].rearrange("p (kt f) -> p kt f", f=E),
            wo_d[:].rearrange("(kt p) f -> p kt f", p=128),
        )
        wo_big = const_p.tile([128, 2 * E], dt.float32r, tag="wo")
        self.round_copy(wo_big[:], wo_st[:])
        wo_r = [wo_big[:, E * kt : E * (kt + 1)] for kt in range(2)]
        wstage_cm.__exit__(None, None, None)

        bqk_sb = const_p.tile([128, 4], dt.float32, tag="bqk")
        nc.sync.dma_start(bqk_sb[:], bqk_d[:].rearrange("(f p) -> p f", p=128))
        bv_sb = const_p.tile([128, 2], dt.float32, tag="bv")
        nc.sync.dma_start(bv_sb[:], bv_d[:].rearrange("(f p) -> p f", p=128))
        ones_sb = const_p.tile([128, 1, 1], dt.float32, tag="ones")
        nc.vector.memset(ones_sb[:], 1.0)
        mask_sb = const_p.tile([128, 128], dt.float32, tag="mask")
        nc.sync.dma_start(mask_sb[:], mask_d[:])

        # ---- persistent activations ----
        # qkt tiles: 0: q heads 0,1 | 1: q heads 2,3 | 2: k heads 0,1 | 3: k heads 2,3
        qkt = [qkt_p.tile([128, S], dt.float32r, tag=f"qkt{f}", name=f"qkt{f}") for f in range(4)]
        # vones[t]: [v h0 |1| v h1 |1| v h2 |1| v h3 |1] for token tile t
        vones = [vones_p.tile([128, 4 * 65], dt.float32r, tag=f"v{t}", name=f"v{t}") for t in range(NT)]
        # pair tiles: final normalized attn output, [head dims x 2, S]
        pairt = [pair_p.tile([128, S], dt.float32r, tag=f"pair{hp}", name=f"pair{hp}") for hp in range(2)]

        env = dict(
            xT_d=xT_d, wqk_r=wqk_r, wv_r=wv_r, bqk_sb=bqk_sb, ones_sb=ones_sb,
            xs_p=xs_p, xr_p=xr_p, p1_p=p1_p, qkt=qkt, vones=vones,
            ps_p=ps_p, po_p=po_p, attn_p=attn_p, small_p=small_p,
            pairt=pairt, bv_sb=bv_sb, mask_sb=mask_sb, wo_r=wo_r,
            y_p=y_p, y_d=y_d,
        )
        # startup: chunk-0 qkv emitted directly
        for u in self.qkv_units(0, env):
            u()
        for c in range(NCH):
            fillers = []
            if c + 1 < NCH:
                fillers += self.qkv_units(c + 1, env)
            tail = self.oproj_units(c - 1, env) if c >= 1 else []
            self.attention_chunk(c, env, fillers, tail)
        for u in self.oproj_units(NCH - 1, env):
            u()

    # ------------------------------------------------------------------
    def qkv_units(self, c, env):
        nc = self.nc
        cs = slice(512 * c, 512 * (c + 1))
        xT_d, wqk_r, wv_r = env["xT_d"], env["wqk_r"], env["wv_r"]
        qkt, vones = env["qkt"], env["vones"]
        bqk_sb, ones_sb = env["bqk_sb"], env["ones_sb"]
        xs_p, xr_p, p1_p = env["xs_p"], env["xr_p"], env["p1_p"]

        xs = xs_p.tile([128, KE * 512], dt.float32, tag="xs", name="xs")
        nc.sync.dma_start(
            xs[:].rearrange("p (ke f) -> p ke f", f=512),
            xT_d[:, cs].rearrange("(ke p) f -> p ke f", p=128),
        )
        xrb = xr_p.tile([128, KE * 512], dt.float32r, tag="xr", name="xrb")
        xr = [xrb[:, 512 * ke : 512 * (ke + 1)] for ke in range(KE)]
        units = []
        for ke in range(KE):
            units.append(lambda ke=ke: self.round_copy(
                xrb[:, 512 * ke : 512 * (ke + 1)], xs[:, 512 * ke : 512 * (ke + 1)]))

        def qk_unit(f):
            pq = p1_p.tile([128, 1024], dt.float32, tag="ps", name="pq")
            for ke in range(KE):
                nc.tensor.matmul(
                    pq[:, 0:512], wqk_r[ke][:, 128 * f : 128 * (f + 1)], xr[ke][:],
                    start=(ke == 0), stop=(ke == KE - 1),
                )
            nc.vector.tensor_scalar_add(qkt[f][:, cs], pq[:, 0:512], bqk_sb[:, f : f + 1])

        def v_unit(t4):
            t = 4 * c + t4
            pv = p1_p.tile([128, 1024], dt.float32, tag="ps", name="pv")
            for ke in range(KE):
                nc.tensor.matmul(
                    pv[:, 0:FV],
                    xr[ke][:, 128 * t4 : 128 * (t4 + 1)], wv_r[ke][:],
                    start=(ke == 0), stop=(ke == KE - 1),
                )
            vt = vones[t]
            v3 = vt[:].rearrange("p (g d) -> p g d", d=65)
            nc.vector.tensor_copy(
                v3[:, :, 0:64],
                pv[:, 0:FV].rearrange("p (g d) -> p g d", d=64),
            )
            nc.vector.tensor_copy(v3[:, :, 64:65], ones_sb[:].to_broadcast((128, 4, 1)))

        for f in range(4):
            units.append(lambda f=f: qk_unit(f))
        for t4 in range(4):
            units.append(lambda t4=t4: v_unit(t4))
        return units

    # ------------------------------------------------------------------
    def oproj_units(self, c, env):
        nc = self.nc
        pairt, wo_r, p1_p, y_p, y_d = (
            env["pairt"], env["wo_r"], env["p1_p"], env["y_p"], env["y_d"])
        units = []
        ysbs = {}

        def unit(t4, o):
            t = 4 * c + t4
            if o == 0:
                ysbs[t4] = y_p.tile([128, E], dt.float32, tag="y", name="ysb")
            ysb = ysbs[t4]
            py = p1_p.tile([128, 1024], dt.float32, tag="ps", name="py")
            for kt in range(2):
                nc.tensor.matmul(
                    py[:, 0:512],
                    pairt[kt][:, 128 * t : 128 * (t + 1)],
                    wo_r[kt][:, 512 * o : 512 * (o + 1)],
                    start=(kt == 0), stop=(kt == 1),
                )
            if o == 0:
                nc.vector.tensor_copy(ysb[:, 0:512], py[:, 0:512])
            else:
                nc.scalar.activation(
                    ysb[:, 512:1024], py[:, 0:512], mybir.ActivationFunctionType.Copy
                )
                eng = nc.gpsimd if t % 2 == 0 else nc.sync
                eng.dma_start(y_d[128 * t : 128 * (t + 1), :], ysb[:])

        for t4 in range(4):
            for o in range(2):
                units.append(lambda t4=t4, o=o: unit(t4, o))
        return units

    # ------------------------------------------------------------------
    def attention_chunk(self, c, env, fillers, tail=()):
        """Attention for both head pairs of chunk c, weaving filler units
        (next-chunk qkv / prev-chunk out-proj) into the PE stream."""
        nc = self.nc
        qkt, vones = env["qkt"], env["vones"]
        ps_p, po_p, attn_p, small_p = (
            env["ps_p"], env["po_p"], env["attn_p"], env["small_p"])
        pairt, bv_sb, mask_sb = env["pairt"], env["bv_sb"], env["mask_sb"]
        nj = 4 * c + 4
        # po[2*hp + h_idx]: [65, 512] accumulator per head
        po = [po_p.tile([65, 512], dt.float32, tag=f"po{i}", name=f"po{i}")
              for i in range(4)]

        nfill = len(fillers)
        iters = 2 * nj
        emitted = 0

        def emit_pv(hp, j, off, at):
            for h_idx in range(2):
                slot = 2 * hp + h_idx
                nc.tensor.matmul(
                    po[slot][:, off:512],
                    vones[j][:, 65 * slot : 65 * slot + 65],
                    at[:, 512 * h_idx + off : 512 * (h_idx + 1)],
                    start=(j == 0), stop=(j == nj - 1),
                    skip_group_check=True,
                )

        it = 0
        for hp in range(2):
            pending = []
            for j in range(nj):
                ps = ps_p.tile([128, 1024], dt.float32, tag="ps", name="ps")
                at = attn_p.tile([128, 1024], dt.float32r, tag="attn", name="at")
                m = j - 4 * c
                off = 128 * m if 1 <= m <= 3 else 0
                off_mm = off if m in (1, 2) else 0
                for h_idx in range(2):
                    r0 = 64 * h_idx
                    nc.tensor.matmul(
                        ps[:, 512 * h_idx + off_mm : 512 * (h_idx + 1)],
                        qkt[2 + hp][r0 : r0 + 64, 128 * j : 128 * (j + 1)],
                        qkt[hp][r0 : r0 + 64, 512 * c + off_mm : 512 * (c + 1)],
                        start=True, stop=True,
                    )
                if m >= 0:
                    for h_idx in range(2):
                        lo = 512 * h_idx + 128 * m
                        nc.vector.tensor_add(
                            ps[:, lo : lo + 128], ps[:, lo : lo + 128], mask_sb[:])
                if off == 0:
                    runs = [(0, 1024)]
                else:
                    runs = [(off, 512), (512 + off, 1024)]
                for lo, hi in runs:
                    nc.scalar.activation(
                        at[:, lo:hi], ps[:, lo:hi], mybir.ActivationFunctionType.Exp)
                pending.append((j, off, at))
                if len(pending) > 2:
                    emit_pv(hp, *pending.pop(0))
                it += 1
                while emitted < nfill and emitted * iters < it * nfill:
                    fillers[emitted]()
                    emitted += 1
            for p in pending:
                emit_pv(hp, *p)
        while emitted < nfill:
            fillers[emitted]()
            emitted += 1
        # ---- batched rollout for both head pairs ----
        recip4 = small_p.tile([128, 512], dt.float32, tag="recip4", name="recip4")
        nc.vector.memset(recip4[:], 1.0)
        for i in range(4):
            nc.vector.tensor_copy(recip4[32 * i : 32 * i + 1, :], po[i][64:65, :])
        nc.vector.reciprocal(recip4[:], recip4[:])
        for hp in range(2):
            bch = small_p.tile([128, 512], dt.float32, tag=f"bc{hp}", name=f"bc{hp}")
            for h_idx in range(2):
                i = 2 * hp + h_idx
                nc.sync.dma_start(
                    bch[64 * h_idx : 64 * h_idx + 64, :],
                    recip4[32 * i : 32 * i + 1, :]
                    .rearrange("a (o n) -> a o n", o=1)
                    .to_broadcast((1, 64, 512)),
                )
            tmp = small_p.tile([128, 512], dt.float32, tag=f"tmp{hp}", name=f"tmp{hp}")
            nc.vector.tensor_mul(tmp[0:64, :], po[2 * hp][0:64, :], bch[0:64, :])
            nc.vector.tensor_mul(tmp[64:128, :], po[2 * hp + 1][0:64, :], bch[64:128, :])
            nc.vector.tensor_scalar_add(
                pairt[hp][:, 512 * c : 512 * (c + 1)], tmp[:], bv_sb[:, hp : hp + 1]
            )
        for u in tail:
            u()

# ----------------------------------------------------------------------
_PROGRAM = None


def _get_program():
    global _PROGRAM
    if _PROGRAM is None:
        _PROGRAM = _build_program()
    return _PROGRAM


def _make_in_maps(inputs, W_in, b_in, W_out, b_out):
    in_maps = []
    scale = 1.0 / np.sqrt(np.float32(HD))
    kr = np.arange(128)[:, None]
    qc = np.arange(128)[None, :]
    trimask = np.where(qc >= kr, 0.0, -1e30).astype(np.float32)
    for core in range(NC):
        b, g = divmod(core, 4)
        r = slice(256 * g, 256 * (g + 1))
        wq = W_in[0:E][r] * scale
        wk = W_in[E : 2 * E][r]
        wv = W_in[2 * E : 3 * E][r]
        xT = np.ascontiguousarray(inputs[b].T.astype(np.float32))
        wqkT = np.ascontiguousarray(np.concatenate([wq, wk], axis=0).T)
        wvT = np.ascontiguousarray(wv.T)
        bqk = np.concatenate([b_in[0:E][r] * scale, b_in[E : 2 * E][r]])
        bv = np.ascontiguousarray(b_in[2 * E : 3 * E][r])
        wo = np.ascontiguousarray(W_out[:, r].T)
        in_maps.append(
            {
                "xT": xT,
                "wqkT": wqkT.astype(np.float32),
                "wvT": wvT.astype(np.float32),
                "bqk": bqk.astype(np.float32),
                "bv": bv.astype(np.float32),
                "wo": wo.astype(np.float32),
                "trimask": trimask,
            }
        )
    return in_maps


def run_spmd(inputs, W_in, b_in, W_out, b_out, trace=False, **kw):
    nc = _get_program()
    in_maps = _make_in_maps(inputs, W_in, b_in, W_out, b_out)
    bkr = run_bass_kernel_spmd(nc, in_maps, list(range(NC)), trace=trace, **kw)
    parts = [bkr.results[i]["y"] for i in range(NC)]
    out = np.stack(
        [
            parts[0] + parts[1] + parts[2] + parts[3],
            parts[4] + parts[5] + parts[6] + parts[7],
        ]
    )
    out = out + b_out[None, None, :]
    return out.astype(np.float32), bkr


def kernel(inputs, W_in, b_in, W_out, b_out):
    out, _ = run_spmd(
        np.asarray(inputs, dtype=np.float32),
        np.asarray(W_in, dtype=np.float32),
        np.asarray(b_in, dtype=np.float32),
        np.asarray(W_out, dtype=np.float32),
        np.asarray(b_out, dtype=np.float32),
    )
    return out



# revision 13
# speedup vs baseline: 1.0656x; 1.0656x over previous
"""Trainium2 Bass kernel for causal multi-head attention (B=2, S=2048, E=1024, H=16).

Sharding: 8 cores = 2 batches x 4 head-groups (4 heads each).
Each core computes its batch's QKV for its 4 heads, causal attention, and a
partial output projection; host sums the 4 group partials per batch, then adds
b_out and the (softmax-invariant-factored) W_out @ b_v term.

All matmul operands are bf16 (same PE rate as fp32r at 1 cyc/row, but valid at
any moving size, FWL weight loads, and half the DVE/DMA traffic).  PSUM stays
fp32.  Score matmuls have K=64 so the two heads of a pair run concurrently in
distinct PE row-groups.  k-bias is dropped (softmax-invariant), v-bias folded
into the host-side output add.
"""
import sys

sys.path.insert(0, "/opt/trn_rl_repo")

from contextlib import ExitStack

import ml_dtypes
import numpy as np

import concourse.bass as bass
import concourse.tile as tile
from concourse import bacc, mybir
from concourse.bass_utils import run_bass_kernel_spmd

dt = mybir.dt
AF = mybir.ActivationFunctionType

B, S, E, H = 2, 2048, 1024, 16
HD = 64                     # head dim
HPC = 4                     # heads per core
NC = 8                      # cores
KE = E // 128               # 8 contraction k-tiles for projections
NT = S // 128               # 16 token tiles
NCH = S // 512              # 4 token chunks
PEND = 4                    # pv emission delay (iterations)



DEBUG_OUTS = False


def _build_program():
    nc = bacc.Bacc("TRN2", target_bir_lowering=False, debug=False, num_devices=NC)

    xT_d = nc.dram_tensor("xT", [E, S], dt.bfloat16, kind="ExternalInput")
    wqkT_d = nc.dram_tensor("wqkT", [E, 512], dt.bfloat16, kind="ExternalInput")
    wvT_d = nc.dram_tensor("wvT", [E, 256], dt.bfloat16, kind="ExternalInput")
    bq_d = nc.dram_tensor("bq", [256], dt.float32, kind="ExternalInput")
    wo_d = nc.dram_tensor("wo", [256, E], dt.bfloat16, kind="ExternalInput")
    mask_d = nc.dram_tensor("trimask", [128, 128], dt.float32, kind="ExternalInput")
    y_d = nc.dram_tensor("y", [S, E], dt.bfloat16, kind="ExternalOutput")

    dbg = {}
    if DEBUG_OUTS:
        dbg["qkt"] = nc.dram_tensor("dbg_qkt", [4, 128, S], dt.bfloat16, kind="ExternalOutput")
        dbg["pair"] = nc.dram_tensor("dbg_pair", [2, 128, S], dt.bfloat16, kind="ExternalOutput")
        dbg["den"] = nc.dram_tensor("dbg_den", [NCH, 2048], dt.float32, kind="ExternalOutput")
        dbg["recip"] = nc.dram_tensor("dbg_recip", [NCH, 2048], dt.bfloat16, kind="ExternalOutput")
        dbg["bcs"] = nc.dram_tensor("dbg_bcs", [NCH, 128, 1024], dt.bfloat16, kind="ExternalOutput")
        dbg["vones"] = nc.dram_tensor("dbg_vones", [NT, 128, 260], dt.bfloat16, kind="ExternalOutput")

    with TileKernel(nc) as tk:
        tk.dbg = dbg
        tk.build(xT_d, wqkT_d, wvT_d, bq_d, wo_d, mask_d, y_d)
    nc.compile()
    return nc


class TileKernel:
    def __init__(self, nc):
        self.nc = nc
        self.dbg = {}
        self.ctx = ExitStack()
        self.tc_cm = tile.TileContext(nc)

    def __enter__(self):
        self.tc = self.tc_cm.__enter__()
        return self

    def __exit__(self, *a):
        self.ctx.close()
        return self.tc_cm.__exit__(*a)

    def build(self, xT_d, wqkT_d, wvT_d, bq_d, wo_d, mask_d, y_d):
        nc, tc, ctx = self.nc, self.tc, self.ctx
        pool = lambda name, bufs, **kw: ctx.enter_context(
            tc.tile_pool(name=name, bufs=bufs, **kw)
        )

        const_p = pool("const", 1)
        xs_p = pool("xs", 1)
        qkt_p = pool("qkt", 1)
        vones_p = pool("vones", 1)
        attn_p = pool("attn", PEND + 2)
        pair_p = pool("pair", 1)
        small_p = pool("small", 2)
        y_p = pool("y", 3)
        ps_p = pool("ps", 2, space="PSUM")     # 2 x [128,1024] = 4 banks
        po_p = pool("po", 1, space="PSUM")     # [65, 4, 512]   = 4 banks

        # ---- small consts first (fast DMAs) ----
        mask_sb = const_p.tile([128, 128], dt.float32, tag="mask")
        nc.sync.dma_start(mask_sb[:], mask_d[:])
        bq_sb = const_p.tile([128, 2], dt.float32, tag="bq")
        nc.sync.dma_start(bq_sb[:], bq_d[:].rearrange("(f p) -> p f", p=128))

        # ---- weights + x, interleaved per-ke so chunk-0 QKV starts early ----
        wqk_sb = const_p.tile([128, KE, 512], dt.bfloat16, tag="wqk")
        xs = xs_p.tile([128, KE, S], dt.bfloat16, tag="xs", name="xs")
        for ke in range(KE):
            nc.sync.dma_start(
                wqk_sb[:, ke, :],
                wqkT_d[128 * ke : 128 * (ke + 1), :],
            )
            nc.sync.dma_start(
                xs[:, ke, 0:512],
                xT_d[128 * ke : 128 * (ke + 1), 0:512],
            )
        wv_sb = const_p.tile([128, KE, 256], dt.bfloat16, tag="wv")
        nc.sync.dma_start(
            wv_sb[:],
            wvT_d[:].rearrange("(ke p) f -> p ke f", p=128),
        )
        for c in range(1, NCH):
            if c == 2:
                wo_sb = const_p.tile([128, 2, E], dt.bfloat16, tag="wo")
                nc.sync.dma_start(
                    wo_sb[:],
                    wo_d[:].rearrange("(kt p) f -> p kt f", p=128),
                )
            cs = slice(512 * c, 512 * (c + 1))
            nc.sync.dma_start(
                xs[:, :, cs],
                xT_d[:, cs].rearrange("(ke p) f -> p ke f", p=128),
            )

        # ---- persistent activations ----
        # qkt tiles: 0: q heads 0,1 | 1: q heads 2,3 | 2: k heads 0,1 | 3: k heads 2,3
        qkt = [qkt_p.tile([128, S], dt.bfloat16, tag=f"qkt{f}", name=f"qkt{f}")
               for f in range(4)]
        # vones[t]: per head [v(64) | 1] -> [128, 4, 65]
        vones = [vones_p.tile([128, 4, 65], dt.bfloat16, tag=f"v{t}", name=f"v{t}")
                 for t in range(NT)]
        for t in range(NT):
            nc.vector.memset(vones[t][:, :, 64:65], 1.0)
        # pairt[kt]: normalized attn output, [2 heads x 64 dims, S]
        pairt = [pair_p.tile([128, S], dt.bfloat16, tag=f"pair{hp}", name=f"pair{hp}")
                 for hp in range(2)]

        env = dict(
            xs=xs, wqk_sb=wqk_sb, wv_sb=wv_sb, bq_sb=bq_sb, wo_sb=wo_sb,
            mask_sb=mask_sb, qkt=qkt, vones=vones, pairt=pairt,
            xs_p=xs_p, ps_p=ps_p, po_p=po_p, attn_p=attn_p, small_p=small_p,
            y_p=y_p, y_d=y_d, po={}, recip={},
        )

        # startup: chunk-0 qkv emitted directly
        for u in self.qkv_units(0, env):
            u()
        for c in range(NCH):
            fillers = []
            if c == 1:
                fillers += self.r2_units(0, env) + self.oproj_units(0, env)
                fillers += self.qkv_units(2, env)
            elif c == 2:
                fillers += self.r2_units(1, env) + self.qkv_units(3, env)
            elif c == 3:
                fillers += self.r2_units(2, env)
                fillers += self.oproj_units(1, env) + self.oproj_units(2, env)
            elif c == 0:
                fillers += self.qkv_units(1, env)
            self.attention_chunk(c, env, fillers)
        for u in self.r2_units(NCH - 1, env):
            u()
        for u in self.oproj_units(NCH - 1, env):
            u()
        if self.dbg:
            for f in range(4):
                nc.sync.dma_start(self.dbg["qkt"][f], qkt[f][:])
            for hp in range(2):
                nc.sync.dma_start(self.dbg["pair"][hp], pairt[hp][:])
            for t in range(NT):
                nc.sync.dma_start(
                    self.dbg["vones"][t],
                    vones[t][:].rearrange("p g d -> p (g d)"),
                )

    # ------------------------------------------------------------------
    def qkv_units(self, c, env):
        nc = self.nc
        cs = slice(512 * c, 512 * (c + 1))
        xs, wqk_sb, wv_sb = env["xs"], env["wqk_sb"], env["wv_sb"]
        bq_sb, qkt, vones = env["bq_sb"], env["qkt"], env["vones"]
        ps_p = env["ps_p"]
        units = []

        def qk_unit(f):
            pq = ps_p.tile([128, 1024], dt.float32, tag="ps", name="pq")
            for ke in range(KE):
                nc.tensor.matmul(
                    pq[:, 0:512],
                    wqk_sb[:, ke, 128 * f : 128 * (f + 1)],
                    xs[:, ke, cs],
                    start=(ke == 0), stop=(ke == KE - 1),
                )
            if f < 2:
                nc.vector.tensor_scalar_add(
                    qkt[f][:, cs], pq[:, 0:512], bq_sb[:, f : f + 1])
            else:
                nc.vector.tensor_copy(qkt[f][:, cs], pq[:, 0:512])

        def v_unit(t4):
            t = 4 * c + t4
            pv = ps_p.tile([128, 1024], dt.float32, tag="ps", name="pv")
            for ke in range(KE):
                nc.tensor.matmul(
                    pv[:, 0:256],
                    xs[:, ke, 512 * c + 128 * t4 : 512 * c + 128 * (t4 + 1)],
                    wv_sb[:, ke, :],
                    start=(ke == 0), stop=(ke == KE - 1),
                )
            nc.vector.tensor_copy(
                vones[t][:, :, 0:64],
                pv[:, 0:256].rearrange("p (g d) -> p g d", d=64),
            )

        for f in range(4):
            units.append(lambda f=f: qk_unit(f))
        for t4 in range(4):
            units.append(lambda t4=t4: v_unit(t4))
        return units

    # ------------------------------------------------------------------
    def r2_units(self, c, env):
        """Broadcast reciprocal denominators and write normalized pairt."""
        nc = self.nc
        ps_p, pairt = env["ps_p"], env["pairt"]
        cs = slice(512 * c, 512 * (c + 1))
        bcs = {}

        def bc_unit():
            recip_bf = env["recip"][c]
            sb = env["small_p"].tile([128, 1024], dt.bfloat16, tag="bcs", name="bcs")
            for hp in range(2):
                for h in range(2):
                    i = 2 * hp + h
                    nc.sync.dma_start(
                        sb[64 * h : 64 * (h + 1), 512 * hp : 512 * (hp + 1)],
                        recip_bf[0:1, 512 * i : 512 * (i + 1)]
                        .rearrange("a (o n) -> a o n", o=1)
                        .to_broadcast((1, 64, 512)),
                    )
            if self.dbg:
                nc.sync.dma_start(self.dbg["bcs"][c], sb[:])
            bcs["bc"] = sb

        def mult_unit(hp):
            po = env["po"][c]
            bc = bcs["bc"]
            for h in range(2):
                nc.vector.tensor_mul(
                    pairt[hp][64 * h : 64 * (h + 1), cs],
                    po[0:64, 2 * hp + h, :],
                    bc[64 * h : 64 * (h + 1), 512 * hp : 512 * (hp + 1)],
                )

        return [bc_unit, lambda: mult_unit(0), lambda: mult_unit(1)]

    # ------------------------------------------------------------------
    def oproj_units(self, c, env):
        nc = self.nc
        pairt, wo_sb, ps_p, y_p, y_d = (
            env["pairt"], env["wo_sb"], env["ps_p"], env["y_p"], env["y_d"])
        units = []

        def unit(t4):
            t = 4 * c + t4
            py = ps_p.tile([128, 1024], dt.float32, tag="ps", name="py")
            for o in range(2):
                for kt in range(2):
                    nc.tensor.matmul(
                        py[:, 512 * o : 512 * (o + 1)],
                        pairt[kt][:, 128 * t : 128 * (t + 1)],
                        wo_sb[:, kt, 512 * o : 512 * (o + 1)],
                        start=(kt == 0), stop=(kt == 1),
                    )
            ysb = y_p.tile([128, E], dt.bfloat16, tag="y", name="ysb")
            nc.vector.tensor_copy(ysb[:, 0:512], py[:, 0:512])
            nc.scalar.activation(ysb[:, 512:1024], py[:, 512:1024], AF.Copy)
            nc.sync.dma_start(y_d[128 * t : 128 * (t + 1), :], ysb[:])

        for t4 in range(4):
            units.append(lambda t4=t4: unit(t4))
        return units

    # ------------------------------------------------------------------
    def attention_chunk(self, c, env, fillers):
        """Attention for both head pairs of chunk c, weaving filler units
        (prev-chunk rollout/out-proj, next-chunk qkv) into the PE stream."""
        nc = self.nc
        qkt, vones, mask_sb = env["qkt"], env["vones"], env["mask_sb"]
        ps_p, po_p, attn_p = env["ps_p"], env["po_p"], env["attn_p"]
        nj = 4 * c + 4
        po = po_p.tile([65, 4, 512], dt.float32, tag="po", name="po")
        env["po"][c] = po

        nfill = len(fillers)
        iters = 2 * nj
        emitted = 0

        def emit_pv(hp, j, off, at):
            for h in range(2):
                i = 2 * hp + h
                nc.tensor.matmul(
                    po[:, i, off:512],
                    vones[j][:, i, :],
                    at[:, 512 * h + off : 512 * (h + 1)],
                    start=(j == 0), stop=(j == nj - 1),
                    skip_group_check=True,
                )

        it = 0
        for hp in range(2):
            pending = []
            for j in range(nj):
                ps = ps_p.tile([128, 1024], dt.float32, tag="ps", name="ps")
                at = attn_p.tile([128, 1024], dt.bfloat16, tag="attn", name="at")
                m = j - 4 * c
                off = 128 * m if m >= 1 else 0
                for h in range(2):
                    r0 = 64 * h
                    nc.tensor.matmul(
                        ps[:, 512 * h + off : 512 * (h + 1)],
                        qkt[2 + hp][r0 : r0 + 64, 128 * j : 128 * (j + 1)],
                        qkt[hp][r0 : r0 + 64, 512 * c + off : 512 * (c + 1)],
                        start=True, stop=True,
                    )
                ps3 = ps[:].rearrange("p (h q) -> p h q", h=2)
                at3 = at[:].rearrange("p (h q) -> p h q", h=2)
                if m >= 0:
                    nc.vector.tensor_add(
                        ps3[:, :, 128 * m : 128 * (m + 1)],
                        ps3[:, :, 128 * m : 128 * (m + 1)],
                        mask_sb[:].rearrange("p (o q) -> p o q", o=1)
                        .to_broadcast((128, 2, 128)),
                    )
                if off == 0:
                    nc.scalar.activation(at[:], ps[:], AF.Exp)
                else:
                    nc.scalar.activation(
                        at3[:, :, off:512], ps3[:, :, off:512], AF.Exp)
                pending.append((j, off, at))
                if len(pending) > PEND:
                    emit_pv(hp, *pending.pop(0))
                it += 1
                while emitted < nfill and emitted * iters < it * nfill:
                    fillers[emitted]()
                    emitted += 1
            for p in pending:
                emit_pv(hp, *p)
        while emitted < nfill:
            fillers[emitted]()
            emitted += 1

        # ---- r1: reciprocal of the 4 denominator rows ----
        recip_f = env["small_p"].tile([1, 2048], dt.float32, tag="recf", name="recf")
        nc.vector.reciprocal(recip_f[0:1, :], po[64:65, :, :])
        recip_bf = env["small_p"].tile([1, 2048], dt.bfloat16, tag="recb", name="recb")
        nc.vector.tensor_copy(recip_bf[0:1, :], recip_f[0:1, :])
        env["recip"][c] = recip_bf
        if self.dbg:
            den_sb = env["small_p"].tile([1, 2048], dt.float32, tag="dens", name="dens")
            nc.vector.tensor_copy(den_sb[0:1, :], po[64:65, :, :])
            nc.sync.dma_start(self.dbg["den"][c], den_sb[0:1, :].rearrange("a n -> (a n)"))
            nc.sync.dma_start(self.dbg["recip"][c], recip_bf[0:1, :].rearrange("a n -> (a n)"))


# ----------------------------------------------------------------------
_PROGRAM = None


def _get_program():
    global _PROGRAM
    if _PROGRAM is None:
        _PROGRAM = _build_program()
    return _PROGRAM


def _make_in_maps(inputs, W_in, b_in, W_out, b_out):
    in_maps = []
    bf16 = ml_dtypes.bfloat16
    scale = 1.0 / np.sqrt(np.float32(HD))
    kr = np.arange(128)[:, None]
    qc = np.arange(128)[None, :]
    trimask = np.where(qc >= kr, 0.0, -1e30).astype(np.float32)
    for core in range(NC):
        b, g = divmod(core, 4)
        r = slice(256 * g, 256 * (g + 1))
        wq = W_in[0:E][r] * scale
        wk = W_in[E : 2 * E][r]
        wv = W_in[2 * E : 3 * E][r]
        xT = np.ascontiguousarray(inputs[b].T).astype(bf16)
        wqkT = np.ascontiguousarray(np.concatenate([wq, wk], axis=0).T).astype(bf16)
        wvT = np.ascontiguousarray(wv.T).astype(bf16)
        bq = (b_in[0:E][r] * scale).astype(np.float32)
        wo = np.ascontiguousarray(W_out[:, r].T).astype(bf16)
        in_maps.append(
            {
                "xT": xT,
                "wqkT": wqkT,
                "wvT": wvT,
                "bq": bq,
                "wo": wo,
                "trimask": trimask,
            }
        )
    return in_maps


def run_spmd(inputs, W_in, b_in, W_out, b_out, trace=False, **kw):
    nc = _get_program()
    in_maps = _make_in_maps(inputs, W_in, b_in, W_out, b_out)
    bkr = run_bass_kernel_spmd(nc, in_maps, list(range(NC)), trace=trace, **kw)
    parts = [bkr.results[i]["y"].astype(np.float32) for i in range(NC)]
    out = np.stack(
        [
            parts[0] + parts[1] + parts[2] + parts[3],
            parts[4] + parts[5] + parts[6] + parts[7],
        ]
    )
    yb = W_out.astype(np.float32) @ b_in[2 * E : 3 * E].astype(np.float32)
    out = out + (yb + b_out)[None, None, :]
    return out.astype(np.float32), bkr


def kernel(inputs, W_in, b_in, W_out, b_out):
    out, _ = run_spmd(
        np.asarray(inputs, dtype=np.float32),
        np.asarray(W_in, dtype=np.float32),
        np.asarray(b_in, dtype=np.float32),
        np.asarray(W_out, dtype=np.float32),
        np.asarray(b_out, dtype=np.float32),
    )
    return out


# revision 16
# speedup vs baseline: 1.0724x; 1.0064x over previous
"""Trainium2 Bass kernel for causal multi-head attention (B=2, S=2048, E=1024, H=16).

Sharding: 8 cores = 2 batches x 4 head-groups (4 heads each).
Each core computes its batch's QKV for its 4 heads, causal attention, and a
partial output projection; host sums the 4 group partials per batch, then adds
b_out and the (softmax-invariant-factored) W_out @ b_v term.

All matmul operands are bf16 (same PE rate as fp32r at 1 cyc/row, but valid at
any moving size, FWL weight loads, and half the DVE/DMA traffic).  PSUM stays
fp32.  Score matmuls have K=64 so the two heads of a pair run concurrently in
distinct PE row-groups.  k-bias is dropped (softmax-invariant), v-bias folded
into the host-side output add.
"""
import sys

sys.path.insert(0, "/opt/trn_rl_repo")

from contextlib import ExitStack

import ml_dtypes
import numpy as np

import concourse.bass as bass
import concourse.tile as tile
from concourse import bacc, mybir
from concourse.bass_utils import run_bass_kernel_spmd

dt = mybir.dt
AF = mybir.ActivationFunctionType

B, S, E, H = 2, 2048, 1024, 16
HD = 64                     # head dim
HPC = 4                     # heads per core
NC = 8                      # cores
KE = E // 128               # 8 contraction k-tiles for projections
NT = S // 128               # 16 token tiles
NCH = S // 512              # 4 token chunks
PEND = 4                    # pv emission delay (iterations)



DEBUG_OUTS = False


def _build_program():
    nc = bacc.Bacc("TRN2", target_bir_lowering=False, debug=False, num_devices=NC)

    xT_d = nc.dram_tensor("xT", [E, S], dt.bfloat16, kind="ExternalInput")
    wqkT_d = nc.dram_tensor("wqkT", [E, 512], dt.bfloat16, kind="ExternalInput")
    wvT_d = nc.dram_tensor("wvT", [E, 256], dt.bfloat16, kind="ExternalInput")
    bq_d = nc.dram_tensor("bq", [256], dt.float32, kind="ExternalInput")
    wo_d = nc.dram_tensor("wo", [256, E], dt.bfloat16, kind="ExternalInput")
    mask_d = nc.dram_tensor("trimask", [128, 128], dt.float32, kind="ExternalInput")
    y_d = nc.dram_tensor("y", [S, E], dt.bfloat16, kind="ExternalOutput")

    dbg = {}
    if DEBUG_OUTS:
        dbg["qkt"] = nc.dram_tensor("dbg_qkt", [4, 128, S], dt.bfloat16, kind="ExternalOutput")
        dbg["pair"] = nc.dram_tensor("dbg_pair", [2, 128, S], dt.bfloat16, kind="ExternalOutput")
        dbg["den"] = nc.dram_tensor("dbg_den", [NCH, 2048], dt.float32, kind="ExternalOutput")
        dbg["recip"] = nc.dram_tensor("dbg_recip", [NCH, 2048], dt.bfloat16, kind="ExternalOutput")
        dbg["bcs"] = nc.dram_tensor("dbg_bcs", [NCH, 128, 1024], dt.bfloat16, kind="ExternalOutput")
        dbg["vones"] = nc.dram_tensor("dbg_vones", [NT, 128, 260], dt.bfloat16, kind="ExternalOutput")

    with TileKernel(nc) as tk:
        tk.dbg = dbg
        tk.build(xT_d, wqkT_d, wvT_d, bq_d, wo_d, mask_d, y_d)
    nc.compile()
    return nc


class TileKernel:
    def __init__(self, nc):
        self.nc = nc
        self.dbg = {}
        self.ctx = ExitStack()
        self.tc_cm = tile.TileContext(nc)

    def __enter__(self):
        self.tc = self.tc_cm.__enter__()
        return self

    def __exit__(self, *a):
        self.ctx.close()
        return self.tc_cm.__exit__(*a)

    def build(self, xT_d, wqkT_d, wvT_d, bq_d, wo_d, mask_d, y_d):
        nc, tc, ctx = self.nc, self.tc, self.ctx
        pool = lambda name, bufs, **kw: ctx.enter_context(
            tc.tile_pool(name=name, bufs=bufs, **kw)
        )

        const_p = pool("const", 1)
        xs_p = pool("xs", 1)
        qkt_p = pool("qkt", 1)
        vones_p = pool("vones", 1)
        attn_p = pool("attn", PEND + 2)
        pair_p = pool("pair", 1)
        small_p = pool("small", 2)
        y_p = pool("y", 3)
        ps_p = pool("ps", 2, space="PSUM")     # 2 x [128,1024] = 4 banks
        po_p = pool("po", 1, space="PSUM")     # [65, 4, 512]   = 4 banks

        # ---- small consts first (fast DMAs) ----
        mask_sb = const_p.tile([128, 128], dt.float32, tag="mask")
        nc.sync.dma_start(mask_sb[:], mask_d[:])
        bq_sb = const_p.tile([128, 2], dt.float32, tag="bq")
        nc.sync.dma_start(bq_sb[:], bq_d[:].rearrange("(f p) -> p f", p=128))

        # ---- weights + x, interleaved per-ke so chunk-0 QKV starts early ----
        wqk_sb = const_p.tile([128, KE, 512], dt.bfloat16, tag="wqk")
        xs = xs_p.tile([128, KE, S], dt.bfloat16, tag="xs", name="xs")
        for ke in range(KE):
            nc.sync.dma_start(
                wqk_sb[:, ke, :],
                wqkT_d[128 * ke : 128 * (ke + 1), :],
            )
            nc.sync.dma_start(
                xs[:, ke, 0:512],
                xT_d[128 * ke : 128 * (ke + 1), 0:512],
            )
        wv_sb = const_p.tile([128, KE, 256], dt.bfloat16, tag="wv")
        nc.sync.dma_start(
            wv_sb[:],
            wvT_d[:].rearrange("(ke p) f -> p ke f", p=128),
        )
        for c in range(1, NCH):
            if c == 2:
                wo_sb = const_p.tile([128, 2, E], dt.bfloat16, tag="wo")
                nc.sync.dma_start(
                    wo_sb[:],
                    wo_d[:].rearrange("(kt p) f -> p kt f", p=128),
                )
            cs = slice(512 * c, 512 * (c + 1))
            nc.sync.dma_start(
                xs[:, :, cs],
                xT_d[:, cs].rearrange("(ke p) f -> p ke f", p=128),
            )

        # ---- persistent activations ----
        # qkt tiles: 0: q heads 0,1 | 1: q heads 2,3 | 2: k heads 0,1 | 3: k heads 2,3
        qkt = [qkt_p.tile([128, S], dt.bfloat16, tag=f"qkt{f}", name=f"qkt{f}")
               for f in range(4)]
        # vones[t]: per head [v(64) | 1] -> [128, 4, 65]
        vones = [vones_p.tile([128, 4, 65], dt.bfloat16, tag=f"v{t}", name=f"v{t}")
                 for t in range(NT)]
        for t in range(NT):
            nc.vector.memset(vones[t][:, :, 64:65], 1.0)
        # pairt[kt]: normalized attn output, [2 heads x 64 dims, S]
        pairt = [pair_p.tile([128, S], dt.bfloat16, tag=f"pair{hp}", name=f"pair{hp}")
                 for hp in range(2)]

        env = dict(
            xs=xs, wqk_sb=wqk_sb, wv_sb=wv_sb, bq_sb=bq_sb, wo_sb=wo_sb,
            mask_sb=mask_sb, qkt=qkt, vones=vones, pairt=pairt,
            xs_p=xs_p, ps_p=ps_p, po_p=po_p, attn_p=attn_p, small_p=small_p,
            y_p=y_p, y_d=y_d, po={}, recip={},
        )

        # startup: chunk-0 qkv emitted directly
        for u in self.qkv_units(0, env):
            u()
        for c in range(NCH):
            fillers = []
            if c == 1:
                fillers += self.r2_units(0, env) + self.oproj_units(0, env)
                fillers += self.qkv_units(2, env)
            elif c == 2:
                fillers += self.r2_units(1, env) + self.qkv_units(3, env)
            elif c == 3:
                fillers += self.r2_units(2, env)
                fillers += self.oproj_units(1, env) + self.oproj_units(2, env)
            elif c == 0:
                fillers += self.qkv_units(1, env)
            self.attention_chunk(c, env, fillers)
        for u in self.r2_units(NCH - 1, env):
            u()
        for u in self.oproj_units(NCH - 1, env):
            u()
        if self.dbg:
            for f in range(4):
                nc.sync.dma_start(self.dbg["qkt"][f], qkt[f][:])
            for hp in range(2):
                nc.sync.dma_start(self.dbg["pair"][hp], pairt[hp][:])
            for t in range(NT):
                nc.sync.dma_start(
                    self.dbg["vones"][t],
                    vones[t][:].rearrange("p g d -> p (g d)"),
                )

    # ------------------------------------------------------------------
    def qkv_units(self, c, env):
        nc = self.nc
        cs = slice(512 * c, 512 * (c + 1))
        xs, wqk_sb, wv_sb = env["xs"], env["wqk_sb"], env["wv_sb"]
        bq_sb, qkt, vones = env["bq_sb"], env["qkt"], env["vones"]
        ps_p = env["ps_p"]
        units = []

        def qk_unit(f):
            pq = ps_p.tile([128, 1024], dt.float32, tag="ps", name="pq")
            for ke in range(KE):
                nc.tensor.matmul(
                    pq[:, 0:512],
                    wqk_sb[:, ke, 128 * f : 128 * (f + 1)],
                    xs[:, ke, cs],
                    start=(ke == 0), stop=(ke == KE - 1),
                )
            if f < 2:
                nc.vector.tensor_scalar_add(
                    qkt[f][:, cs], pq[:, 0:512], bq_sb[:, f : f + 1])
            else:
                nc.vector.tensor_copy(qkt[f][:, cs], pq[:, 0:512])

        def v_unit(t4):
            t = 4 * c + t4
            pv = ps_p.tile([128, 1024], dt.float32, tag="ps", name="pv")
            for ke in range(KE):
                nc.tensor.matmul(
                    pv[:, 0:256],
                    xs[:, ke, 512 * c + 128 * t4 : 512 * c + 128 * (t4 + 1)],
                    wv_sb[:, ke, :],
                    start=(ke == 0), stop=(ke == KE - 1),
                )
            nc.vector.tensor_copy(
                vones[t][:, :, 0:64],
                pv[:, 0:256].rearrange("p (g d) -> p g d", d=64),
            )

        for f in range(4):
            units.append(lambda f=f: qk_unit(f))
        for t4 in range(4):
            units.append(lambda t4=t4: v_unit(t4))
        return units

    # ------------------------------------------------------------------
    def r2_units(self, c, env):
        """Broadcast reciprocal denominators and write normalized pairt."""
        nc = self.nc
        ps_p, pairt = env["ps_p"], env["pairt"]
        cs = slice(512 * c, 512 * (c + 1))
        bcs = {}

        def bc_unit():
            recip_bf = env["recip"][c]
            sb = env["small_p"].tile([128, 1024], dt.bfloat16, tag="bcs", name="bcs")
            for hp in range(2):
                for h in range(2):
                    i = 2 * hp + h
                    nc.sync.dma_start(
                        sb[64 * h : 64 * (h + 1), 512 * hp : 512 * (hp + 1)],
                        recip_bf[0:1, 512 * i : 512 * (i + 1)]
                        .rearrange("a (o n) -> a o n", o=1)
                        .to_broadcast((1, 64, 512)),
                    )
            if self.dbg:
                nc.sync.dma_start(self.dbg["bcs"][c], sb[:])
            bcs["bc"] = sb

        def mult_unit(hp):
            bc = bcs["bc"]
            for h in range(2):
                sl = pairt[hp][64 * h : 64 * (h + 1), cs]
                nc.gpsimd.tensor_mul(
                    sl, sl,
                    bc[64 * h : 64 * (h + 1), 512 * hp : 512 * (hp + 1)],
                )

        return [bc_unit, lambda: mult_unit(0), lambda: mult_unit(1)]

    # ------------------------------------------------------------------
    def oproj_units(self, c, env):
        nc = self.nc
        pairt, wo_sb, ps_p, y_p, y_d = (
            env["pairt"], env["wo_sb"], env["ps_p"], env["y_p"], env["y_d"])
        units = []

        def unit(t4):
            t = 4 * c + t4
            py = ps_p.tile([128, 1024], dt.float32, tag="ps", name="py")
            for o in range(2):
                for kt in range(2):
                    nc.tensor.matmul(
                        py[:, 512 * o : 512 * (o + 1)],
                        pairt[kt][:, 128 * t : 128 * (t + 1)],
                        wo_sb[:, kt, 512 * o : 512 * (o + 1)],
                        start=(kt == 0), stop=(kt == 1),
                    )
            ysb = y_p.tile([128, E], dt.bfloat16, tag="y", name="ysb")
            nc.vector.tensor_copy(ysb[:], py[:])
            nc.sync.dma_start(y_d[128 * t : 128 * (t + 1), :], ysb[:])

        for t4 in range(4):
            units.append(lambda t4=t4: unit(t4))
        return units

    # ------------------------------------------------------------------
    def attention_chunk(self, c, env, fillers):
        """Attention for both head pairs of chunk c, weaving filler units
        (prev-chunk rollout/out-proj, next-chunk qkv) into the PE stream."""
        nc = self.nc
        qkt, vones, mask_sb = env["qkt"], env["vones"], env["mask_sb"]
        ps_p, po_p, attn_p = env["ps_p"], env["po_p"], env["attn_p"]
        nj = 4 * c + 4
        po = po_p.tile([65, 4, 512], dt.float32, tag="po", name="po")
        env["po"][c] = po

        nfill = len(fillers)
        iters = 2 * nj
        emitted = 0

        def emit_pv(hp, j, off, at):
            for h in range(2):
                i = 2 * hp + h
                nc.tensor.matmul(
                    po[:, i, off:512],
                    vones[j][:, i, :],
                    at[:, 512 * h + off : 512 * (h + 1)],
                    start=(j == 0), stop=(j == nj - 1),
                    skip_group_check=True,
                )

        it = 0
        for hp in range(2):
            pending = []
            for j in range(nj):
                ps = ps_p.tile([128, 1024], dt.float32, tag="ps", name="ps")
                at = attn_p.tile([128, 1024], dt.bfloat16, tag="attn", name="at")
                m = j - 4 * c
                off = 128 * m if m >= 1 else 0
                for h in range(2):
                    r0 = 64 * h
                    nc.tensor.matmul(
                        ps[:, 512 * h + off : 512 * (h + 1)],
                        qkt[2 + hp][r0 : r0 + 64, 128 * j : 128 * (j + 1)],
                        qkt[hp][r0 : r0 + 64, 512 * c + off : 512 * (c + 1)],
                        start=True, stop=True,
                    )
                ps3 = ps[:].rearrange("p (h q) -> p h q", h=2)
                at3 = at[:].rearrange("p (h q) -> p h q", h=2)
                if m >= 0:
                    nc.vector.tensor_add(
                        ps3[:, :, 128 * m : 128 * (m + 1)],
                        ps3[:, :, 128 * m : 128 * (m + 1)],
                        mask_sb[:].rearrange("p (o q) -> p o q", o=1)
                        .to_broadcast((128, 2, 128)),
                    )
                if off == 0:
                    nc.scalar.activation(at[:], ps[:], AF.Exp)
                else:
                    nc.scalar.activation(
                        at3[:, :, off:512], ps3[:, :, off:512], AF.Exp)
                pending.append((j, off, at))
                if len(pending) > PEND:
                    emit_pv(hp, *pending.pop(0))
                it += 1
                while emitted < nfill and emitted * iters < it * nfill:
                    fillers[emitted]()
                    emitted += 1
            for p in pending:
                emit_pv(hp, *p)
        while emitted < nfill:
            fillers[emitted]()
            emitted += 1

        # ---- r1: unnormalized pairt copy (frees po fast) + 1/denom via
        # exp(-ln(d)) on the scalar engine (same ACT table as softmax exp) ----
        pairt = env["pairt"]
        cs = slice(512 * c, 512 * (c + 1))
        for hp in range(2):
            for h in range(2):
                nc.vector.tensor_copy(
                    pairt[hp][64 * h : 64 * (h + 1), cs],
                    po[0:64, 2 * hp + h, :],
                )
        lnrow = env["small_p"].tile([1, 2048], dt.float32, tag="recf", name="lnrow")
        nc.scalar.activation(lnrow[0:1, :], po[64:65, :, :], AF.Ln)
        recip_bf = env["small_p"].tile([1, 2048], dt.bfloat16, tag="recb", name="recb")
        nc.scalar.activation(recip_bf[0:1, :], lnrow[0:1, :], AF.Exp, scale=-1.0)
        env["recip"][c] = recip_bf
        if self.dbg:
            den_sb = env["small_p"].tile([1, 2048], dt.float32, tag="dens", name="dens")
            nc.vector.tensor_copy(den_sb[0:1, :], po[64:65, :, :])
            nc.sync.dma_start(self.dbg["den"][c], den_sb[0:1, :].rearrange("a n -> (a n)"))
            nc.sync.dma_start(self.dbg["recip"][c], recip_bf[0:1, :].rearrange("a n -> (a n)"))


# ----------------------------------------------------------------------
_PROGRAM = None


def _get_program():
    global _PROGRAM
    if _PROGRAM is None:
        _PROGRAM = _build_program()
    return _PROGRAM


def _make_in_maps(inputs, W_in, b_in, W_out, b_out):
    in_maps = []
    bf16 = ml_dtypes.bfloat16
    scale = 1.0 / np.sqrt(np.float32(HD))
    kr = np.arange(128)[:, None]
    qc = np.arange(128)[None, :]
    trimask = np.where(qc >= kr, 0.0, -1e30).astype(np.float32)
    for core in range(NC):
        b, g = divmod(core, 4)
        r = slice(256 * g, 256 * (g + 1))
        wq = W_in[0:E][r] * scale
        wk = W_in[E : 2 * E][r]
        wv = W_in[2 * E : 3 * E][r]
        xT = np.ascontiguousarray(inputs[b].T).astype(bf16)
        wqkT = np.ascontiguousarray(np.concatenate([wq, wk], axis=0).T).astype(bf16)
        wvT = np.ascontiguousarray(wv.T).astype(bf16)
        bq = (b_in[0:E][r] * scale).astype(np.float32)
        wo = np.ascontiguousarray(W_out[:, r].T).astype(bf16)
        in_maps.append(
            {
                "xT": xT,
                "wqkT": wqkT,
                "wvT": wvT,
                "bq": bq,
                "wo": wo,
                "trimask": trimask,
            }
        )
    return in_maps


def run_spmd(inputs, W_in, b_in, W_out, b_out, trace=False, **kw):
    nc = _get_program()
    in_maps = _make_in_maps(inputs, W_in, b_in, W_out, b_out)
    bkr = run_bass_kernel_spmd(nc, in_maps, list(range(NC)), trace=trace, **kw)
    parts = [bkr.results[i]["y"].astype(np.float32) for i in range(NC)]
    out = np.stack(
        [
            parts[0] + parts[1] + parts[2] + parts[3],
            parts[4] + parts[5] + parts[6] + parts[7],
        ]
    )
    yb = W_out.astype(np.float32) @ b_in[2 * E : 3 * E].astype(np.float32)
    out = out + (yb + b_out)[None, None, :]
    return out.astype(np.float32), bkr


def kernel(inputs, W_in, b_in, W_out, b_out):
    out, _ = run_spmd(
        np.asarray(inputs, dtype=np.float32),
        np.asarray(W_in, dtype=np.float32),
        np.asarray(b_in, dtype=np.float32),
        np.asarray(W_out, dtype=np.float32),
        np.asarray(b_out, dtype=np.float32),
    )
    return out


# revision 22
# speedup vs baseline: 1.1906x; 1.1103x over previous
"""Trainium2 Bass kernel for causal multi-head attention (B=2, S=2048, E=1024, H=16).

Sharding: 8 cores = 2 batches x 4 head-groups (4 heads each).
Each core computes its batch's QKV for its 4 heads, causal attention, and a
partial output projection; host sums the 4 group partials per batch, then adds
b_out and the (softmax-invariant-factored) W_out @ b_v term.

All matmul operands are bf16 (same PE rate as fp32r at 1 cyc/row, but valid at
any moving size, FWL weight loads, and half the DVE/DMA traffic).  PSUM stays
fp32.  Score matmuls have K=64 so the two heads of a pair run concurrently in
distinct PE row-groups.  k-bias is dropped (softmax-invariant), v-bias folded
into the host-side output add.
"""
import sys

sys.path.insert(0, "/opt/trn_rl_repo")

from contextlib import ExitStack

import ml_dtypes
import numpy as np

import concourse.bass as bass
import concourse.tile as tile
from concourse import bacc, mybir
from concourse.bass_utils import run_bass_kernel_spmd

dt = mybir.dt
AF = mybir.ActivationFunctionType

B, S, E, H = 2, 2048, 1024, 16
HD = 64                     # head dim
HPC = 4                     # heads per core
NC = 8                      # cores
KE = E // 128               # 8 contraction k-tiles for projections
NT = S // 128               # 16 token tiles
NCH = S // 512              # 4 token chunks
PEND = 4                    # pv emission delay (iterations)



DEBUG_OUTS = False


def _build_program():
    nc = bacc.Bacc("TRN2", target_bir_lowering=False, debug=False, num_devices=NC)

    xT_d = nc.dram_tensor("xT", [E, S], dt.bfloat16, kind="ExternalInput")
    wqkT_d = nc.dram_tensor("wqkT", [E, 512], dt.bfloat16, kind="ExternalInput")
    wvT_d = nc.dram_tensor("wvT", [E, 256], dt.bfloat16, kind="ExternalInput")
    bq_d = nc.dram_tensor("bq", [256], dt.float32, kind="ExternalInput")
    wo_d = nc.dram_tensor("wo", [256, E], dt.bfloat16, kind="ExternalInput")
    mask_d = nc.dram_tensor("trimask", [128, 128], dt.float32, kind="ExternalInput")
    y_d = nc.dram_tensor("y", [S, E], dt.bfloat16, kind="ExternalOutput")

    dbg = {}
    if DEBUG_OUTS:
        dbg["qkt"] = nc.dram_tensor("dbg_qkt", [4, 128, S], dt.bfloat16, kind="ExternalOutput")
        dbg["pair"] = nc.dram_tensor("dbg_pair", [2, 128, S], dt.bfloat16, kind="ExternalOutput")
        dbg["den"] = nc.dram_tensor("dbg_den", [NCH, 2048], dt.float32, kind="ExternalOutput")
        dbg["recip"] = nc.dram_tensor("dbg_recip", [NCH, 2048], dt.bfloat16, kind="ExternalOutput")
        dbg["bcs"] = nc.dram_tensor("dbg_bcs", [NCH, 128, 1024], dt.bfloat16, kind="ExternalOutput")
        dbg["vones"] = nc.dram_tensor("dbg_vones", [NT, 128, 260], dt.bfloat16, kind="ExternalOutput")

    with TileKernel(nc) as tk:
        tk.dbg = dbg
        tk.build(xT_d, wqkT_d, wvT_d, bq_d, wo_d, mask_d, y_d)
    nc.compile()
    return nc


class TileKernel:
    def __init__(self, nc):
        self.nc = nc
        self.dbg = {}
        self.ctx = ExitStack()
        self.tc_cm = tile.TileContext(nc)

    def __enter__(self):
        self.tc = self.tc_cm.__enter__()
        return self

    def __exit__(self, *a):
        self.ctx.close()
        return self.tc_cm.__exit__(*a)

    def build(self, xT_d, wqkT_d, wvT_d, bq_d, wo_d, mask_d, y_d):
        nc, tc, ctx = self.nc, self.tc, self.ctx
        pool = lambda name, bufs, **kw: ctx.enter_context(
            tc.tile_pool(name=name, bufs=bufs, **kw)
        )

        const_p = pool("const", 1)
        xs_p = pool("xs", 1)
        qkt_p = pool("qkt", 1)
        vones_p = pool("vones", 1)
        attn_p = pool("attn", PEND + 2)
        pair_p = pool("pair", 1)
        small_p = pool("small", 2)
        y_p = pool("y", 3)
        ps_p = pool("ps", 2, space="PSUM")     # 2 x [128,1024] = 4 banks
        po_p = pool("po", 1, space="PSUM")     # [65, 4, 512]   = 4 banks

        # ---- small consts first (fast DMAs) ----
        mask_sb = const_p.tile([128, 128], dt.float32, tag="mask")
        nc.sync.dma_start(mask_sb[:], mask_d[:])
        bq_sb = const_p.tile([128, 2], dt.float32, tag="bq")
        nc.sync.dma_start(bq_sb[:], bq_d[:].rearrange("(f p) -> p f", p=128))

        # ---- weights + x, interleaved per-ke so chunk-0 QKV starts early ----
        wqk_sb = const_p.tile([128, KE, 512], dt.bfloat16, tag="wqk")
        xs = xs_p.tile([128, KE, S], dt.bfloat16, tag="xs", name="xs")
        for ke in range(KE):
            nc.sync.dma_start(
                wqk_sb[:, ke, :],
                wqkT_d[128 * ke : 128 * (ke + 1), :],
            )
            nc.sync.dma_start(
                xs[:, ke, 0:512],
                xT_d[128 * ke : 128 * (ke + 1), 0:512],
            )
        wv_sb = const_p.tile([128, KE, 256], dt.bfloat16, tag="wv")
        nc.sync.dma_start(
            wv_sb[:],
            wvT_d[:].rearrange("(ke p) f -> p ke f", p=128),
        )
        for c in range(1, NCH):
            if c == 2:
                wo_sb = const_p.tile([128, 2, E], dt.bfloat16, tag="wo")
                nc.sync.dma_start(
                    wo_sb[:],
                    wo_d[:].rearrange("(kt p) f -> p kt f", p=128),
                )
            cs = slice(512 * c, 512 * (c + 1))
            nc.sync.dma_start(
                xs[:, :, cs],
                xT_d[:, cs].rearrange("(ke p) f -> p ke f", p=128),
            )

        # ---- persistent activations ----
        # qkt tiles: 0: q heads 0,1 | 1: q heads 2,3 | 2: k heads 0,1 | 3: k heads 2,3
        qkt = [qkt_p.tile([128, S], dt.bfloat16, tag=f"qkt{f}", name=f"qkt{f}")
               for f in range(4)]
        # vones[t]: per head [v(64) | 1] -> [128, 4, 65]
        vones = [vones_p.tile([128, 4, 65], dt.bfloat16, tag=f"v{t}", name=f"v{t}")
                 for t in range(NT)]
        for t in range(NT):
            nc.vector.memset(vones[t][:, :, 64:65], 1.0)
        # pairt[kt]: normalized attn output, [2 heads x 64 dims, S]
        pairt = [pair_p.tile([128, S], dt.bfloat16, tag=f"pair{hp}", name=f"pair{hp}")
                 for hp in range(2)]

        env = dict(
            xs=xs, wqk_sb=wqk_sb, wv_sb=wv_sb, bq_sb=bq_sb, wo_sb=wo_sb,
            mask_sb=mask_sb, qkt=qkt, vones=vones, pairt=pairt,
            xs_p=xs_p, ps_p=ps_p, po_p=po_p, attn_p=attn_p, small_p=small_p,
            y_p=y_p, y_d=y_d, po={}, recip={},
        )

        # startup: chunk-0 qkv emitted directly
        for u in self.qkv_units(0, env):
            u()
        for c in range(NCH):
            fillers = []
            if c == 1:
                qkv2 = self.qkv_units(2, env)
                fillers += self.r2_units(0, env) + qkv2[:4]
                fillers += self.oproj_units(0, env) + qkv2[4:]
            elif c == 2:
                fillers += self.r2_units(1, env) + self.qkv_units(3, env)
            elif c == 3:
                fillers += self.r2_units(2, env)
                fillers += self.oproj_units(1, env) + self.oproj_units(2, env)
            elif c == 0:
                fillers += self.qkv_units(1, env)
            self.attention_chunk(c, env, fillers)
        for u in self.r2_units(NCH - 1, env):
            u()
        for u in self.oproj_units(NCH - 1, env):
            u()
        if self.dbg:
            for f in range(4):
                nc.sync.dma_start(self.dbg["qkt"][f], qkt[f][:])
            for hp in range(2):
                nc.sync.dma_start(self.dbg["pair"][hp], pairt[hp][:])
            for t in range(NT):
                nc.sync.dma_start(
                    self.dbg["vones"][t],
                    vones[t][:].rearrange("p g d -> p (g d)"),
                )

    # ------------------------------------------------------------------
    def qkv_units(self, c, env):
        nc = self.nc
        cs = slice(512 * c, 512 * (c + 1))
        xs, wqk_sb, wv_sb = env["xs"], env["wqk_sb"], env["wv_sb"]
        bq_sb, qkt, vones = env["bq_sb"], env["qkt"], env["vones"]
        ps_p = env["ps_p"]
        units = []

        def qk_unit(f):
            pq = ps_p.tile([128, 1024], dt.float32, tag="ps", name="pq")
            for ke in range(KE):
                nc.tensor.matmul(
                    pq[:, 0:512],
                    wqk_sb[:, ke, 128 * f : 128 * (f + 1)],
                    xs[:, ke, cs],
                    start=(ke == 0), stop=(ke == KE - 1),
                )
            if f < 2:
                nc.vector.tensor_scalar_add(
                    qkt[f][:, cs], pq[:, 0:512], bq_sb[:, f : f + 1])
            else:
                nc.vector.tensor_copy(qkt[f][:, cs], pq[:, 0:512])

        def v_unit(t4):
            t = 4 * c + t4
            pv = ps_p.tile([128, 1024], dt.float32, tag="ps", name="pv")
            for ke in range(KE):
                nc.tensor.matmul(
                    pv[:, 0:256],
                    xs[:, ke, 512 * c + 128 * t4 : 512 * c + 128 * (t4 + 1)],
                    wv_sb[:, ke, :],
                    start=(ke == 0), stop=(ke == KE - 1),
                )
            nc.vector.tensor_copy(
                vones[t][:, :, 0:64],
                pv[:, 0:256].rearrange("p (g d) -> p g d", d=64),
            )

        for f in range(4):
            units.append(lambda f=f: qk_unit(f))
        for t4 in range(4):
            units.append(lambda t4=t4: v_unit(t4))
        return units

    # ------------------------------------------------------------------
    def r2_units(self, c, env):
        """Broadcast reciprocal denominators and write normalized pairt."""
        nc = self.nc
        ps_p, pairt = env["ps_p"], env["pairt"]
        cs = slice(512 * c, 512 * (c + 1))
        bcs = {}

        def bc_unit():
            recrow = env["recip"][c]
            sb = env["small_p"].tile([128, 1024], dt.float32, tag="bcs", name="bcs")
            for hp in range(2):
                for h in range(2):
                    i = 2 * hp + h
                    nc.sync.dma_start(
                        sb[64 * h : 64 * (h + 1), 512 * hp : 512 * (hp + 1)],
                        recrow[0:1, 512 * i : 512 * (i + 1)]
                        .rearrange("a (o n) -> a o n", o=1)
                        .to_broadcast((1, 64, 512)),
                    )
            bcs["bc"] = sb

        def mult_unit(hp):
            bc = bcs["bc"]
            for h in range(2):
                sl = pairt[hp][64 * h : 64 * (h + 1), cs]
                eng = nc.vector if h == 0 else nc.gpsimd
                eng.tensor_mul(
                    sl, sl,
                    bc[64 * h : 64 * (h + 1), 512 * hp : 512 * (hp + 1)],
                )

        return [bc_unit, lambda: mult_unit(0), lambda: mult_unit(1)]

    # ------------------------------------------------------------------
    def oproj_units(self, c, env):
        nc = self.nc
        pairt, wo_sb, ps_p, y_p, y_d = (
            env["pairt"], env["wo_sb"], env["ps_p"], env["y_p"], env["y_d"])
        units = []

        def unit(t4):
            t = 4 * c + t4
            py = ps_p.tile([128, 1024], dt.float32, tag="ps", name="py")
            for o in range(2):
                for kt in range(2):
                    nc.tensor.matmul(
                        py[:, 512 * o : 512 * (o + 1)],
                        pairt[kt][:, 128 * t : 128 * (t + 1)],
                        wo_sb[:, kt, 512 * o : 512 * (o + 1)],
                        start=(kt == 0), stop=(kt == 1),
                    )
            ysb = y_p.tile([128, E], dt.bfloat16, tag="y", name="ysb")
            nc.vector.tensor_copy(ysb[:], py[:])
            nc.gpsimd.dma_start(y_d[128 * t : 128 * (t + 1), :], ysb[:])

        for t4 in range(4):
            units.append(lambda t4=t4: unit(t4))
        return units

    # ------------------------------------------------------------------
    def attention_chunk(self, c, env, fillers):
        """Attention for both head pairs of chunk c, weaving filler units
        (prev-chunk rollout/out-proj, next-chunk qkv) into the PE stream."""
        nc = self.nc
        qkt, vones, mask_sb = env["qkt"], env["vones"], env["mask_sb"]
        ps_p, po_p, attn_p = env["ps_p"], env["po_p"], env["attn_p"]
        nj = 4 * c + 4
        po = po_p.tile([65, 4, 512], dt.float32, tag="po", name="po")
        env["po"][c] = po

        nfill = len(fillers)
        iters = 2 * nj
        emitted = 0

        def emit_pv(hp, j, off, at):
            for h in range(2):
                i = 2 * hp + h
                nc.tensor.matmul(
                    po[:, i, off:512],
                    vones[j][:, i, :],
                    at[:, 512 * h + off : 512 * (h + 1)],
                    start=(j == 0), stop=(j == nj - 1),
                    skip_group_check=True,
                )

        it = 0
        for hp in range(2):
            pending = []
            for j in range(nj):
                ps = ps_p.tile([128, 1024], dt.float32, tag="ps", name="ps")
                at = attn_p.tile([128, 1024], dt.bfloat16, tag="attn", name="at")
                m = j - 4 * c
                off = 128 * m if m >= 1 else 0
                for h in range(2):
                    r0 = 64 * h
                    nc.tensor.matmul(
                        ps[:, 512 * h + off : 512 * (h + 1)],
                        qkt[2 + hp][r0 : r0 + 64, 128 * j : 128 * (j + 1)],
                        qkt[hp][r0 : r0 + 64, 512 * c + off : 512 * (c + 1)],
                        start=True, stop=True,
                    )
                ps3 = ps[:].rearrange("p (h q) -> p h q", h=2)
                at3 = at[:].rearrange("p (h q) -> p h q", h=2)
                if m >= 0:
                    nc.vector.tensor_add(
                        ps3[:, :, 128 * m : 128 * (m + 1)],
                        ps3[:, :, 128 * m : 128 * (m + 1)],
                        mask_sb[:].rearrange("p (o q) -> p o q", o=1)
                        .to_broadcast((128, 2, 128)),
                    )
                if off == 0:
                    nc.scalar.activation(at[:], ps[:], AF.Exp)
                else:
                    nc.scalar.activation(
                        at3[:, :, off:512], ps3[:, :, off:512], AF.Exp)
                pending.append((j, off, at))
                if len(pending) > PEND:
                    emit_pv(hp, *pending.pop(0))
                it += 1
                while emitted < nfill and emitted * iters < it * nfill:
                    fillers[emitted]()
                    emitted += 1
            for p in pending:
                emit_pv(hp, *p)
        while emitted < nfill:
            fillers[emitted]()
            emitted += 1

        # ---- r1: unnormalized pairt copy (frees po fast) + 1/denom via
        # exp(-ln(d)) on the scalar engine (same ACT table as softmax exp) ----
        pairt = env["pairt"]
        cs = slice(512 * c, 512 * (c + 1))
        for hp in range(2):
            for h in range(2):
                nc.vector.tensor_copy(
                    pairt[hp][64 * h : 64 * (h + 1), cs],
                    po[0:64, 2 * hp + h, :],
                )
        # denominator reciprocal: ACT Copy (no table switch) -> DMA reshape to
        # [128,16] -> wide DVE reciprocal -> DMA back to a row for broadcasting
        denrow = env["small_p"].tile([1, 2048], dt.float32, tag="recf", name="denrow")
        nc.scalar.activation(denrow[0:1, :], po[64:65, :, :], AF.Copy)
        dencol = env["small_p"].tile([128, 16], dt.float32, tag="denc", name="dencol")
        nc.sync.dma_start(dencol[:], denrow[0:1, :])
        reccol = env["small_p"].tile([128, 16], dt.float32, tag="recc", name="reccol")
        nc.vector.reciprocal(reccol[:], dencol[:])
        recrow = env["small_p"].tile([1, 2048], dt.float32, tag="recb", name="recrow")
        nc.sync.dma_start(recrow[0:1, :], reccol[:])
        env["recip"][c] = recrow
        if self.dbg:
            den_sb = env["small_p"].tile([1, 2048], dt.float32, tag="dens", name="dens")
            nc.vector.tensor_copy(den_sb[0:1, :], po[64:65, :, :])
            nc.sync.dma_start(self.dbg["den"][c], den_sb[0:1, :].rearrange("a n -> (a n)"))
            nc.sync.dma_start(self.dbg["recip"][c], recip_bf[0:1, :].rearrange("a n -> (a n)"))


# ----------------------------------------------------------------------
_PROGRAM = None


def _get_program():
    global _PROGRAM
    if _PROGRAM is None:
        _PROGRAM = _build_program()
    return _PROGRAM


def _make_in_maps(inputs, W_in, b_in, W_out, b_out):
    in_maps = []
    bf16 = ml_dtypes.bfloat16
    scale = 1.0 / np.sqrt(np.float32(HD))
    kr = np.arange(128)[:, None]
    qc = np.arange(128)[None, :]
    trimask = np.where(qc >= kr, 0.0, -1e30).astype(np.float32)
    for core in range(NC):
        b, g = divmod(core, 4)
        r = slice(256 * g, 256 * (g + 1))
        wq = W_in[0:E][r] * scale
        wk = W_in[E : 2 * E][r]
        wv = W_in[2 * E : 3 * E][r]
        xT = np.ascontiguousarray(inputs[b].T).astype(bf16)
        wqkT = np.ascontiguousarray(np.concatenate([wq, wk], axis=0).T).astype(bf16)
        wvT = np.ascontiguousarray(wv.T).astype(bf16)
        bq = (b_in[0:E][r] * scale).astype(np.float32)
        wo = np.ascontiguousarray(W_out[:, r].T).astype(bf16)
        in_maps.append(
            {
                "xT": xT,
                "wqkT": wqkT,
                "wvT": wvT,
                "bq": bq,
                "wo": wo,
                "trimask": trimask,
            }
        )
    return in_maps


def run_spmd(inputs, W_in, b_in, W_out, b_out, trace=False, **kw):
    nc = _get_program()
    in_maps = _make_in_maps(inputs, W_in, b_in, W_out, b_out)
    bkr = run_bass_kernel_spmd(nc, in_maps, list(range(NC)), trace=trace, **kw)
    parts = [bkr.results[i]["y"].astype(np.float32) for i in range(NC)]
    out = np.stack(
        [
            parts[0] + parts[1] + parts[2] + parts[3],
            parts[4] + parts[5] + parts[6] + parts[7],
        ]
    )
    yb = W_out.astype(np.float32) @ b_in[2 * E : 3 * E].astype(np.float32)
    out = out + (yb + b_out)[None, None, :]
    return out.astype(np.float32), bkr


def kernel(inputs, W_in, b_in, W_out, b_out):
    out, _ = run_spmd(
        np.asarray(inputs, dtype=np.float32),
        np.asarray(W_in, dtype=np.float32),
        np.asarray(b_in, dtype=np.float32),
        np.asarray(W_out, dtype=np.float32),
        np.asarray(b_out, dtype=np.float32),
    )
    return out


# revision 23
# speedup vs baseline: 1.2013x; 1.0090x over previous
"""Trainium2 Bass kernel for causal multi-head attention (B=2, S=2048, E=1024, H=16).

Sharding: 8 cores = 2 batches x 4 head-groups (4 heads each).
Each core computes its batch's QKV for its 4 heads, causal attention, and a
partial output projection; host sums the 4 group partials per batch, then adds
b_out and the (softmax-invariant-factored) W_out @ b_v term.

All matmul operands are bf16 (same PE rate as fp32r at 1 cyc/row, but valid at
any moving size, FWL weight loads, and half the DVE/DMA traffic).  PSUM stays
fp32.  Score matmuls have K=64 so the two heads of a pair run concurrently in
distinct PE row-groups.  k-bias is dropped (softmax-invariant), v-bias folded
into the host-side output add.
"""
import sys

sys.path.insert(0, "/opt/trn_rl_repo")

from contextlib import ExitStack

import ml_dtypes
import numpy as np

import concourse.bass as bass
import concourse.tile as tile
from concourse import bacc, mybir
from concourse.bass_utils import run_bass_kernel_spmd

dt = mybir.dt
AF = mybir.ActivationFunctionType

B, S, E, H = 2, 2048, 1024, 16
HD = 64                     # head dim
HPC = 4                     # heads per core
NC = 8                      # cores
KE = E // 128               # 8 contraction k-tiles for projections
NT = S // 128               # 16 token tiles
NCH = S // 512              # 4 token chunks
PEND = 4                    # pv emission delay (iterations)



DEBUG_OUTS = False


def _build_program():
    nc = bacc.Bacc("TRN2", target_bir_lowering=False, debug=False, num_devices=NC)

    xT_d = nc.dram_tensor("xT", [E, S], dt.bfloat16, kind="ExternalInput")
    wqkT_d = nc.dram_tensor("wqkT", [E, 512], dt.bfloat16, kind="ExternalInput")
    wvT_d = nc.dram_tensor("wvT", [E, 256], dt.bfloat16, kind="ExternalInput")
    bq_d = nc.dram_tensor("bq", [256], dt.float32, kind="ExternalInput")
    wo_d = nc.dram_tensor("wo", [256, E], dt.bfloat16, kind="ExternalInput")
    mask_d = nc.dram_tensor("trimask", [128, 128], dt.float32, kind="ExternalInput")
    y_d = nc.dram_tensor("y", [S, E], dt.bfloat16, kind="ExternalOutput")

    dbg = {}
    if DEBUG_OUTS:
        dbg["qkt"] = nc.dram_tensor("dbg_qkt", [4, 128, S], dt.bfloat16, kind="ExternalOutput")
        dbg["pair"] = nc.dram_tensor("dbg_pair", [2, 128, S], dt.bfloat16, kind="ExternalOutput")
        dbg["den"] = nc.dram_tensor("dbg_den", [NCH, 2048], dt.float32, kind="ExternalOutput")
        dbg["recip"] = nc.dram_tensor("dbg_recip", [NCH, 2048], dt.bfloat16, kind="ExternalOutput")
        dbg["bcs"] = nc.dram_tensor("dbg_bcs", [NCH, 128, 1024], dt.bfloat16, kind="ExternalOutput")
        dbg["vones"] = nc.dram_tensor("dbg_vones", [NT, 128, 260], dt.bfloat16, kind="ExternalOutput")

    with TileKernel(nc) as tk:
        tk.dbg = dbg
        tk.build(xT_d, wqkT_d, wvT_d, bq_d, wo_d, mask_d, y_d)
    nc.compile()
    return nc


class TileKernel:
    def __init__(self, nc):
        self.nc = nc
        self.dbg = {}
        self.ctx = ExitStack()
        self.tc_cm = tile.TileContext(nc)

    def __enter__(self):
        self.tc = self.tc_cm.__enter__()
        return self

    def __exit__(self, *a):
        self.ctx.close()
        return self.tc_cm.__exit__(*a)

    def build(self, xT_d, wqkT_d, wvT_d, bq_d, wo_d, mask_d, y_d):
        nc, tc, ctx = self.nc, self.tc, self.ctx
        pool = lambda name, bufs, **kw: ctx.enter_context(
            tc.tile_pool(name=name, bufs=bufs, **kw)
        )

        const_p = pool("const", 1)
        xs_p = pool("xs", 1)
        qkt_p = pool("qkt", 1)
        vones_p = pool("vones", 1)
        attn_p = pool("attn", PEND + 2)
        pair_p = pool("pair", 1)
        small_p = pool("small", 2)
        y_p = pool("y", 3)
        ps_p = pool("ps", 2, space="PSUM")     # 2 x [128,1024] = 4 banks
        po_p = pool("po", 1, space="PSUM")     # 2 x [65, 2, 512] = 4 banks

        # ---- small consts first (fast DMAs) ----
        mask_sb = const_p.tile([128, 128], dt.float32, tag="mask")
        nc.sync.dma_start(mask_sb[:], mask_d[:])
        bq_sb = const_p.tile([128, 2], dt.float32, tag="bq")
        nc.sync.dma_start(bq_sb[:], bq_d[:].rearrange("(f p) -> p f", p=128))

        # ---- weights + x, interleaved per-ke so chunk-0 QKV starts early ----
        wqk_sb = const_p.tile([128, KE, 512], dt.bfloat16, tag="wqk")
        xs = xs_p.tile([128, KE, S], dt.bfloat16, tag="xs", name="xs")
        for ke in range(KE):
            nc.sync.dma_start(
                wqk_sb[:, ke, :],
                wqkT_d[128 * ke : 128 * (ke + 1), :],
            )
            nc.sync.dma_start(
                xs[:, ke, 0:512],
                xT_d[128 * ke : 128 * (ke + 1), 0:512],
            )
        wv_sb = const_p.tile([128, KE, 256], dt.bfloat16, tag="wv")
        nc.sync.dma_start(
            wv_sb[:],
            wvT_d[:].rearrange("(ke p) f -> p ke f", p=128),
        )
        for c in range(1, NCH):
            if c == 2:
                wo_sb = const_p.tile([128, 2, E], dt.bfloat16, tag="wo")
                nc.sync.dma_start(
                    wo_sb[:],
                    wo_d[:].rearrange("(kt p) f -> p kt f", p=128),
                )
            cs = slice(512 * c, 512 * (c + 1))
            nc.sync.dma_start(
                xs[:, :, cs],
                xT_d[:, cs].rearrange("(ke p) f -> p ke f", p=128),
            )

        # ---- persistent activations ----
        # qkt tiles: 0: q heads 0,1 | 1: q heads 2,3 | 2: k heads 0,1 | 3: k heads 2,3
        qkt = [qkt_p.tile([128, S], dt.bfloat16, tag=f"qkt{f}", name=f"qkt{f}")
               for f in range(4)]
        # vones[t]: per head [v(64) | 1] -> [128, 4, 65]
        vones = [vones_p.tile([128, 4, 65], dt.bfloat16, tag=f"v{t}", name=f"v{t}")
                 for t in range(NT)]
        for t in range(NT):
            nc.vector.memset(vones[t][:, :, 64:65], 1.0)
        # pairt[kt]: normalized attn output, [2 heads x 64 dims, S]
        pairt = [pair_p.tile([128, S], dt.bfloat16, tag=f"pair{hp}", name=f"pair{hp}")
                 for hp in range(2)]

        env = dict(
            xs=xs, wqk_sb=wqk_sb, wv_sb=wv_sb, bq_sb=bq_sb, wo_sb=wo_sb,
            mask_sb=mask_sb, qkt=qkt, vones=vones, pairt=pairt,
            xs_p=xs_p, ps_p=ps_p, po_p=po_p, attn_p=attn_p, small_p=small_p,
            y_p=y_p, y_d=y_d, po={}, recip={},
        )

        # startup: chunk-0 qkv emitted directly
        for u in self.qkv_units(0, env):
            u()
        for c in range(NCH):
            fillers = []
            if c == 1:
                fillers += self.r2_units(0, env) + self.qkv_units(2, env)
            elif c == 2:
                fillers += self.r2_units(1, env) + self.oproj_units(0, env)
                fillers += self.qkv_units(3, env)
            elif c == 3:
                fillers += self.r2_units(2, env)
                fillers += self.oproj_units(1, env) + self.oproj_units(2, env)
            elif c == 0:
                fillers += self.qkv_units(1, env)
            self.attention_chunk(c, env, fillers)
        for u in self.r2_units(NCH - 1, env):
            u()
        for u in self.oproj_units(NCH - 1, env):
            u()
        if self.dbg:
            for f in range(4):
                nc.sync.dma_start(self.dbg["qkt"][f], qkt[f][:])
            for hp in range(2):
                nc.sync.dma_start(self.dbg["pair"][hp], pairt[hp][:])
            for t in range(NT):
                nc.sync.dma_start(
                    self.dbg["vones"][t],
                    vones[t][:].rearrange("p g d -> p (g d)"),
                )

    # ------------------------------------------------------------------
    def qkv_units(self, c, env):
        nc = self.nc
        cs = slice(512 * c, 512 * (c + 1))
        xs, wqk_sb, wv_sb = env["xs"], env["wqk_sb"], env["wv_sb"]
        bq_sb, qkt, vones = env["bq_sb"], env["qkt"], env["vones"]
        ps_p = env["ps_p"]
        units = []

        def qk_unit(f):
            pq = ps_p.tile([128, 1024], dt.float32, tag="ps", name="pq")
            for ke in range(KE):
                nc.tensor.matmul(
                    pq[:, 0:512],
                    wqk_sb[:, ke, 128 * f : 128 * (f + 1)],
                    xs[:, ke, cs],
                    start=(ke == 0), stop=(ke == KE - 1),
                )
            if f < 2:
                nc.vector.tensor_scalar_add(
                    qkt[f][:, cs], pq[:, 0:512], bq_sb[:, f : f + 1])
            else:
                nc.vector.tensor_copy(qkt[f][:, cs], pq[:, 0:512])

        def v_unit(t4):
            t = 4 * c + t4
            pv = ps_p.tile([128, 1024], dt.float32, tag="ps", name="pv")
            for ke in range(KE):
                nc.tensor.matmul(
                    pv[:, 0:256],
                    xs[:, ke, 512 * c + 128 * t4 : 512 * c + 128 * (t4 + 1)],
                    wv_sb[:, ke, :],
                    start=(ke == 0), stop=(ke == KE - 1),
                )
            nc.vector.tensor_copy(
                vones[t][:, :, 0:64],
                pv[:, 0:256].rearrange("p (g d) -> p g d", d=64),
            )

        for f in range(4):
            units.append(lambda f=f: qk_unit(f))
        for t4 in range(4):
            units.append(lambda t4=t4: v_unit(t4))
        return units

    # ------------------------------------------------------------------
    def r2_units(self, c, env):
        """Broadcast reciprocal denominators and normalize pairt in place."""
        nc = self.nc
        pairt = env["pairt"]
        cs = slice(512 * c, 512 * (c + 1))
        bcs = {}

        def bc_unit(hp):
            recrow = env["recip"][(c, hp)]
            sb = env["small_p"].tile([128, 512], dt.float32, tag=f"bcs{hp}",
                                     name="bcs")
            for h in range(2):
                nc.sync.dma_start(
                    sb[64 * h : 64 * (h + 1), :],
                    recrow[0:1, 512 * h : 512 * (h + 1)]
                    .rearrange("a (o n) -> a o n", o=1)
                    .to_broadcast((1, 64, 512)),
                )
            bcs[hp] = sb

        def mult_unit(hp):
            bc = bcs[hp]
            for h in range(2):
                sl = pairt[hp][64 * h : 64 * (h + 1), cs]
                eng = nc.vector if h == 0 else nc.gpsimd
                eng.tensor_mul(sl, sl, bc[64 * h : 64 * (h + 1), :])

        return [lambda: bc_unit(0), lambda: mult_unit(0),
                lambda: bc_unit(1), lambda: mult_unit(1)]

    # ------------------------------------------------------------------
    def oproj_units(self, c, env):
        nc = self.nc
        pairt, wo_sb, ps_p, y_p, y_d = (
            env["pairt"], env["wo_sb"], env["ps_p"], env["y_p"], env["y_d"])
        units = []

        def unit(t4):
            t = 4 * c + t4
            py = ps_p.tile([128, 1024], dt.float32, tag="ps", name="py")
            for o in range(2):
                for kt in range(2):
                    nc.tensor.matmul(
                        py[:, 512 * o : 512 * (o + 1)],
                        pairt[kt][:, 128 * t : 128 * (t + 1)],
                        wo_sb[:, kt, 512 * o : 512 * (o + 1)],
                        start=(kt == 0), stop=(kt == 1),
                    )
            ysb = y_p.tile([128, E], dt.bfloat16, tag="y", name="ysb")
            nc.vector.tensor_copy(ysb[:], py[:])
            nc.gpsimd.dma_start(y_d[128 * t : 128 * (t + 1), :], ysb[:])

        for t4 in range(4):
            units.append(lambda t4=t4: unit(t4))
        return units

    # ------------------------------------------------------------------
    def attention_chunk(self, c, env, fillers):
        """Attention for both head pairs of chunk c, weaving filler units
        (prev-chunk rollout/out-proj, next-chunk qkv) into the PE stream."""
        nc = self.nc
        qkt, vones, mask_sb = env["qkt"], env["vones"], env["mask_sb"]
        ps_p, po_p, attn_p = env["ps_p"], env["po_p"], env["attn_p"]
        nj = 4 * c + 4
        po_t = [po_p.tile([65, 2, 512], dt.float32, tag=f"po{hp}", name="po")
                for hp in range(2)]

        nfill = len(fillers)
        iters = 2 * nj
        emitted = 0

        def emit_pv(hp, j, off, at):
            for h in range(2):
                i = 2 * hp + h
                nc.tensor.matmul(
                    po_t[hp][:, h, off:512],
                    vones[j][:, i, :],
                    at[:, 512 * h + off : 512 * (h + 1)],
                    start=(j == 0), stop=(j == nj - 1),
                    skip_group_check=True,
                )

        it = 0
        for hp in range(2):
            pending = []
            for j in range(nj):
                ps = ps_p.tile([128, 1024], dt.float32, tag="ps", name="ps")
                at = attn_p.tile([128, 1024], dt.bfloat16, tag="attn", name="at")
                m = j - 4 * c
                off = 128 * m if m >= 1 else 0
                for h in range(2):
                    r0 = 64 * h
                    nc.tensor.matmul(
                        ps[:, 512 * h + off : 512 * (h + 1)],
                        qkt[2 + hp][r0 : r0 + 64, 128 * j : 128 * (j + 1)],
                        qkt[hp][r0 : r0 + 64, 512 * c + off : 512 * (c + 1)],
                        start=True, stop=True,
                    )
                ps3 = ps[:].rearrange("p (h q) -> p h q", h=2)
                at3 = at[:].rearrange("p (h q) -> p h q", h=2)
                if m >= 0:
                    nc.vector.tensor_add(
                        ps3[:, :, 128 * m : 128 * (m + 1)],
                        ps3[:, :, 128 * m : 128 * (m + 1)],
                        mask_sb[:].rearrange("p (o q) -> p o q", o=1)
                        .to_broadcast((128, 2, 128)),
                    )
                if off == 0:
                    nc.scalar.activation(at[:], ps[:], AF.Exp)
                else:
                    nc.scalar.activation(
                        at3[:, :, off:512], ps3[:, :, off:512], AF.Exp)
                pending.append((j, off, at))
                if len(pending) > PEND:
                    emit_pv(hp, *pending.pop(0))
                it += 1
                while emitted < nfill and emitted * iters < it * nfill:
                    fillers[emitted]()
                    emitted += 1
            for p in pending:
                emit_pv(hp, *p)
            self.rollout_hp(c, hp, po_t[hp], env)
        while emitted < nfill:
            fillers[emitted]()
            emitted += 1

    def rollout_hp(self, c, hp, po, env):
        """Copy unnormalized attn output to pairt (freeing po) and compute
        1/denominator: ACT Copy (no table switch) -> DMA reshape to [128,8]
        -> wide DVE reciprocal -> DMA back to a row for broadcasting."""
        nc = self.nc
        pairt = env["pairt"]
        cs = slice(512 * c, 512 * (c + 1))
        for h in range(2):
            nc.vector.tensor_copy(
                pairt[hp][64 * h : 64 * (h + 1), cs], po[0:64, h, :])
        denrow = env["small_p"].tile([1, 1024], dt.float32, tag=f"recf{hp}",
                                     name="denrow")
        nc.scalar.activation(denrow[0:1, :], po[64:65, :, :], AF.Copy)
        dencol = env["small_p"].tile([128, 8], dt.float32, tag=f"denc{hp}",
                                     name="dencol")
        nc.sync.dma_start(dencol[:], denrow[0:1, :])
        reccol = env["small_p"].tile([128, 8], dt.float32, tag=f"recc{hp}",
                                     name="reccol")
        nc.vector.reciprocal(reccol[:], dencol[:])
        recrow = env["small_p"].tile([1, 1024], dt.float32, tag=f"recb{hp}",
                                     name="recrow")
        nc.sync.dma_start(recrow[0:1, :], reccol[:])
        env["recip"][(c, hp)] = recrow
        if self.dbg:
            den_sb = env["small_p"].tile([1, 2048], dt.float32, tag="dens", name="dens")
            nc.vector.tensor_copy(den_sb[0:1, :], po[64:65, :, :])
            nc.sync.dma_start(self.dbg["den"][c], den_sb[0:1, :].rearrange("a n -> (a n)"))
            nc.sync.dma_start(self.dbg["recip"][c], recip_bf[0:1, :].rearrange("a n -> (a n)"))


# ----------------------------------------------------------------------
_PROGRAM = None


def _get_program():
    global _PROGRAM
    if _PROGRAM is None:
        _PROGRAM = _build_program()
    return _PROGRAM


def _make_in_maps(inputs, W_in, b_in, W_out, b_out):
    in_maps = []
    bf16 = ml_dtypes.bfloat16
    scale = 1.0 / np.sqrt(np.float32(HD))
    kr = np.arange(128)[:, None]
    qc = np.arange(128)[None, :]
    trimask = np.where(qc >= kr, 0.0, -1e30).astype(np.float32)
    for core in range(NC):
        b, g = divmod(core, 4)
        r = slice(256 * g, 256 * (g + 1))
        wq = W_in[0:E][r] * scale
        wk = W_in[E : 2 * E][r]
        wv = W_in[2 * E : 3 * E][r]
        xT = np.ascontiguousarray(inputs[b].T).astype(bf16)
        wqkT = np.ascontiguousarray(np.concatenate([wq, wk], axis=0).T).astype(bf16)
        wvT = np.ascontiguousarray(wv.T).astype(bf16)
        bq = (b_in[0:E][r] * scale).astype(np.float32)
        wo = np.ascontiguousarray(W_out[:, r].T).astype(bf16)
        in_maps.append(
            {
                "xT": xT,
                "wqkT": wqkT,
                "wvT": wvT,
                "bq": bq,
                "wo": wo,
                "trimask": trimask,
            }
        )
    return in_maps


def run_spmd(inputs, W_in, b_in, W_out, b_out, trace=False, **kw):
    nc = _get_program()
    in_maps = _make_in_maps(inputs, W_in, b_in, W_out, b_out)
    bkr = run_bass_kernel_spmd(nc, in_maps, list(range(NC)), trace=trace, **kw)
    parts = [bkr.results[i]["y"].astype(np.float32) for i in range(NC)]
    out = np.stack(
        [
            parts[0] + parts[1] + parts[2] + parts[3],
            parts[4] + parts[5] + parts[6] + parts[7],
        ]
    )
    yb = W_out.astype(np.float32) @ b_in[2 * E : 3 * E].astype(np.float32)
    out = out + (yb + b_out)[None, None, :]
    return out.astype(np.float32), bkr


def kernel(inputs, W_in, b_in, W_out, b_out):
    out, _ = run_spmd(
        np.asarray(inputs, dtype=np.float32),
        np.asarray(W_in, dtype=np.float32),
        np.asarray(b_in, dtype=np.float32),
        np.asarray(W_out, dtype=np.float32),
        np.asarray(b_out, dtype=np.float32),
    )
    return out


# revision 24
# speedup vs baseline: 1.2031x; 1.0014x over previous
"""Trainium2 Bass kernel for causal multi-head attention (B=2, S=2048, E=1024, H=16).

Sharding: 8 cores = 2 batches x 4 head-groups (4 heads each).
Each core computes its batch's QKV for its 4 heads, causal attention, and a
partial output projection; host sums the 4 group partials per batch, then adds
b_out and the (softmax-invariant-factored) W_out @ b_v term.

All matmul operands are bf16 (same PE rate as fp32r at 1 cyc/row, but valid at
any moving size, FWL weight loads, and half the DVE/DMA traffic).  PSUM stays
fp32.  Score matmuls have K=64 so the two heads of a pair run concurrently in
distinct PE row-groups.  k-bias is dropped (softmax-invariant), v-bias folded
into the host-side output add.
"""
import sys

sys.path.insert(0, "/opt/trn_rl_repo")

from contextlib import ExitStack

import ml_dtypes
import numpy as np

import concourse.bass as bass
import concourse.tile as tile
from concourse import bacc, mybir
from concourse.bass_utils import run_bass_kernel_spmd

dt = mybir.dt
AF = mybir.ActivationFunctionType

B, S, E, H = 2, 2048, 1024, 16
HD = 64                     # head dim
HPC = 4                     # heads per core
NC = 8                      # cores
KE = E // 128               # 8 contraction k-tiles for projections
NT = S // 128               # 16 token tiles
NCH = S // 512              # 4 token chunks
PEND = 4                    # pv emission delay (iterations)



DEBUG_OUTS = False


def _build_program():
    nc = bacc.Bacc("TRN2", target_bir_lowering=False, debug=False, num_devices=NC)

    xT_d = nc.dram_tensor("xT", [E, S], dt.bfloat16, kind="ExternalInput")
    wqkT_d = nc.dram_tensor("wqkT", [E, 512], dt.bfloat16, kind="ExternalInput")
    wvT_d = nc.dram_tensor("wvT", [E, 256], dt.bfloat16, kind="ExternalInput")
    bq_d = nc.dram_tensor("bq", [256], dt.float32, kind="ExternalInput")
    wo_d = nc.dram_tensor("wo", [256, E], dt.bfloat16, kind="ExternalInput")
    mask_d = nc.dram_tensor("trimask", [128, 128], dt.float32, kind="ExternalInput")
    y_d = nc.dram_tensor("y", [S, E], dt.bfloat16, kind="ExternalOutput")

    dbg = {}
    if DEBUG_OUTS:
        dbg["qkt"] = nc.dram_tensor("dbg_qkt", [4, 128, S], dt.bfloat16, kind="ExternalOutput")
        dbg["pair"] = nc.dram_tensor("dbg_pair", [2, 128, S], dt.bfloat16, kind="ExternalOutput")
        dbg["den"] = nc.dram_tensor("dbg_den", [NCH, 2048], dt.float32, kind="ExternalOutput")
        dbg["recip"] = nc.dram_tensor("dbg_recip", [NCH, 2048], dt.bfloat16, kind="ExternalOutput")
        dbg["bcs"] = nc.dram_tensor("dbg_bcs", [NCH, 128, 1024], dt.bfloat16, kind="ExternalOutput")
        dbg["vones"] = nc.dram_tensor("dbg_vones", [NT, 128, 260], dt.bfloat16, kind="ExternalOutput")

    with TileKernel(nc) as tk:
        tk.dbg = dbg
        tk.build(xT_d, wqkT_d, wvT_d, bq_d, wo_d, mask_d, y_d)
    nc.compile()
    return nc


class TileKernel:
    def __init__(self, nc):
        self.nc = nc
        self.dbg = {}
        self.ctx = ExitStack()
        self.tc_cm = tile.TileContext(nc)

    def __enter__(self):
        self.tc = self.tc_cm.__enter__()
        return self

    def __exit__(self, *a):
        self.ctx.close()
        return self.tc_cm.__exit__(*a)

    def build(self, xT_d, wqkT_d, wvT_d, bq_d, wo_d, mask_d, y_d):
        nc, tc, ctx = self.nc, self.tc, self.ctx
        pool = lambda name, bufs, **kw: ctx.enter_context(
            tc.tile_pool(name=name, bufs=bufs, **kw)
        )

        const_p = pool("const", 1)
        xs_p = pool("xs", 1)
        qkt_p = pool("qkt", 1)
        vones_p = pool("vones", 1)
        attn_p = pool("attn", PEND + 2)
        pair_p = pool("pair", 1)
        small_p = pool("small", 2)
        y_p = pool("y", 3)
        ps_p = pool("ps", 2, space="PSUM")     # 2 x [128,1024] = 4 banks
        po_p = pool("po", 1, space="PSUM")     # 2 x [65, 2, 512] = 4 banks

        # ---- small consts first (fast DMAs) ----
        mask_sb = const_p.tile([128, 128], dt.float32, tag="mask")
        nc.sync.dma_start(mask_sb[:], mask_d[:])
        bq_sb = const_p.tile([128, 2], dt.float32, tag="bq")
        nc.sync.dma_start(bq_sb[:], bq_d[:].rearrange("(f p) -> p f", p=128))

        # ---- weights + x, interleaved per-ke so chunk-0 QKV starts early ----
        wqk_sb = const_p.tile([128, KE, 512], dt.bfloat16, tag="wqk")
        xs = xs_p.tile([128, KE, S], dt.bfloat16, tag="xs", name="xs")
        for ke in range(KE):
            nc.sync.dma_start(
                wqk_sb[:, ke, :],
                wqkT_d[128 * ke : 128 * (ke + 1), :],
            )
            nc.sync.dma_start(
                xs[:, ke, 0:512],
                xT_d[128 * ke : 128 * (ke + 1), 0:512],
            )
        wv_sb = const_p.tile([128, KE, 256], dt.bfloat16, tag="wv")
        nc.sync.dma_start(
            wv_sb[:],
            wvT_d[:].rearrange("(ke p) f -> p ke f", p=128),
        )
        for c in range(1, NCH):
            if c == 2:
                wo_sb = const_p.tile([128, 2, E], dt.bfloat16, tag="wo")
                nc.sync.dma_start(
                    wo_sb[:],
                    wo_d[:].rearrange("(kt p) f -> p kt f", p=128),
                )
            cs = slice(512 * c, 512 * (c + 1))
            nc.sync.dma_start(
                xs[:, :, cs],
                xT_d[:, cs].rearrange("(ke p) f -> p ke f", p=128),
            )

        # ---- persistent activations ----
        # qkt tiles: 0: q heads 0,1 | 1: q heads 2,3 | 2: k heads 0,1 | 3: k heads 2,3
        qkt = [qkt_p.tile([128, S], dt.bfloat16, tag=f"qkt{f}", name=f"qkt{f}")
               for f in range(4)]
        # vones[t]: per head [v(64) | 1] -> [128, 4, 65]
        vones = [vones_p.tile([128, 4, 65], dt.bfloat16, tag=f"v{t}", name=f"v{t}")
                 for t in range(NT)]
        for t in range(NT):
            nc.vector.memset(vones[t][:, :, 64:65], 1.0)
        # pairt[kt]: normalized attn output, [2 heads x 64 dims, S]
        pairt = [pair_p.tile([128, S], dt.bfloat16, tag=f"pair{hp}", name=f"pair{hp}")
                 for hp in range(2)]

        env = dict(
            xs=xs, wqk_sb=wqk_sb, wv_sb=wv_sb, bq_sb=bq_sb, wo_sb=wo_sb,
            mask_sb=mask_sb, qkt=qkt, vones=vones, pairt=pairt,
            xs_p=xs_p, ps_p=ps_p, po_p=po_p, attn_p=attn_p, small_p=small_p,
            y_p=y_p, y_d=y_d, po={}, recip={},
        )

        # startup: chunk-0 qkv emitted directly
        for u in self.qkv_units(0, env):
            u()
        for c in range(NCH):
            fillers = []
            if c == 1:
                fillers += self.r2_units(0, env) + self.qkv_units(2, env)
            elif c == 2:
                fillers += self.r2_units(1, env) + self.oproj_units(0, env)
                fillers += self.qkv_units(3, env)
            elif c == 3:
                fillers += self.r2_units(2, env)
                fillers += self.oproj_units(1, env) + self.oproj_units(2, env)
            elif c == 0:
                fillers += self.qkv_units(1, env)
            self.attention_chunk(c, env, fillers)
        if self.dbg:
            for f in range(4):
                nc.sync.dma_start(self.dbg["qkt"][f], qkt[f][:])
            for hp in range(2):
                nc.sync.dma_start(self.dbg["pair"][hp], pairt[hp][:])
            for t in range(NT):
                nc.sync.dma_start(
                    self.dbg["vones"][t],
                    vones[t][:].rearrange("p g d -> p (g d)"),
                )

    # ------------------------------------------------------------------
    def qkv_units(self, c, env):
        nc = self.nc
        cs = slice(512 * c, 512 * (c + 1))
        xs, wqk_sb, wv_sb = env["xs"], env["wqk_sb"], env["wv_sb"]
        bq_sb, qkt, vones = env["bq_sb"], env["qkt"], env["vones"]
        ps_p = env["ps_p"]
        units = []

        def qk_unit(f):
            pq = ps_p.tile([128, 1024], dt.float32, tag="ps", name="pq")
            for ke in range(KE):
                nc.tensor.matmul(
                    pq[:, 0:512],
                    wqk_sb[:, ke, 128 * f : 128 * (f + 1)],
                    xs[:, ke, cs],
                    start=(ke == 0), stop=(ke == KE - 1),
                )
            if f < 2:
                nc.vector.tensor_scalar_add(
                    qkt[f][:, cs], pq[:, 0:512], bq_sb[:, f : f + 1])
            else:
                nc.vector.tensor_copy(qkt[f][:, cs], pq[:, 0:512])

        def v_unit(t4):
            t = 4 * c + t4
            pv = ps_p.tile([128, 1024], dt.float32, tag="ps", name="pv")
            for ke in range(KE):
                nc.tensor.matmul(
                    pv[:, 0:256],
                    xs[:, ke, 512 * c + 128 * t4 : 512 * c + 128 * (t4 + 1)],
                    wv_sb[:, ke, :],
                    start=(ke == 0), stop=(ke == KE - 1),
                )
            nc.vector.tensor_copy(
                vones[t][:, :, 0:64],
                pv[:, 0:256].rearrange("p (g d) -> p g d", d=64),
            )

        for f in range(4):
            units.append(lambda f=f: qk_unit(f))
        for t4 in range(4):
            units.append(lambda t4=t4: v_unit(t4))
        return units

    # ------------------------------------------------------------------
    def r2_units(self, c, env):
        """Broadcast reciprocal denominators and normalize pairt in place."""
        nc = self.nc
        pairt = env["pairt"]
        cs = slice(512 * c, 512 * (c + 1))
        bcs = {}

        def bc_unit(hp):
            recrow = env["recip"][(c, hp)]
            sb = env["small_p"].tile([128, 512], dt.float32, tag=f"bcs{hp}",
                                     name="bcs")
            for h in range(2):
                nc.sync.dma_start(
                    sb[64 * h : 64 * (h + 1), :],
                    recrow[0:1, 512 * h : 512 * (h + 1)]
                    .rearrange("a (o n) -> a o n", o=1)
                    .to_broadcast((1, 64, 512)),
                )
            bcs[hp] = sb

        def mult_unit(hp):
            bc = bcs[hp]
            for h in range(2):
                sl = pairt[hp][64 * h : 64 * (h + 1), cs]
                eng = nc.vector if h == 0 else nc.gpsimd
                eng.tensor_mul(sl, sl, bc[64 * h : 64 * (h + 1), :])

        return [lambda: bc_unit(0), lambda: mult_unit(0),
                lambda: bc_unit(1), lambda: mult_unit(1)]

    # ------------------------------------------------------------------
    def oproj_units(self, c, env):
        nc = self.nc
        pairt, wo_sb, ps_p, y_p, y_d = (
            env["pairt"], env["wo_sb"], env["ps_p"], env["y_p"], env["y_d"])
        units = []

        def unit(t4):
            t = 4 * c + t4
            py = ps_p.tile([128, 1024], dt.float32, tag="ps", name="py")
            for o in range(2):
                for kt in range(2):
                    nc.tensor.matmul(
                        py[:, 512 * o : 512 * (o + 1)],
                        pairt[kt][:, 128 * t : 128 * (t + 1)],
                        wo_sb[:, kt, 512 * o : 512 * (o + 1)],
                        start=(kt == 0), stop=(kt == 1),
                    )
            ysb = y_p.tile([128, E], dt.bfloat16, tag="y", name="ysb")
            nc.vector.tensor_copy(ysb[:], py[:])
            nc.gpsimd.dma_start(y_d[128 * t : 128 * (t + 1), :], ysb[:])

        for t4 in range(4):
            units.append(lambda t4=t4: unit(t4))
        return units

    # ------------------------------------------------------------------
    def attention_chunk(self, c, env, fillers):
        """Attention for both head pairs of chunk c, weaving filler units
        (prev-chunk rollout/out-proj, next-chunk qkv) into the PE stream."""
        nc = self.nc
        qkt, vones, mask_sb = env["qkt"], env["vones"], env["mask_sb"]
        ps_p, po_p, attn_p = env["ps_p"], env["po_p"], env["attn_p"]
        nj = 4 * c + 4
        po_t = [po_p.tile([65, 2, 512], dt.float32, tag=f"po{hp}", name="po")
                for hp in range(2)]

        nfill = len(fillers)
        iters = 2 * nj
        emitted = 0

        def emit_pv(hp, j, off, at):
            for h in range(2):
                i = 2 * hp + h
                nc.tensor.matmul(
                    po_t[hp][:, h, off:512],
                    vones[j][:, i, :],
                    at[:, 512 * h + off : 512 * (h + 1)],
                    start=(j == 0), stop=(j == nj - 1),
                    skip_group_check=True,
                )

        it = 0
        for hp in range(2):
            pending = []
            for j in range(nj):
                ps = ps_p.tile([128, 1024], dt.float32, tag="ps", name="ps")
                at = attn_p.tile([128, 1024], dt.bfloat16, tag="attn", name="at")
                m = j - 4 * c
                off = 128 * m if m >= 1 else 0
                for h in range(2):
                    r0 = 64 * h
                    nc.tensor.matmul(
                        ps[:, 512 * h + off : 512 * (h + 1)],
                        qkt[2 + hp][r0 : r0 + 64, 128 * j : 128 * (j + 1)],
                        qkt[hp][r0 : r0 + 64, 512 * c + off : 512 * (c + 1)],
                        start=True, stop=True,
                    )
                ps3 = ps[:].rearrange("p (h q) -> p h q", h=2)
                at3 = at[:].rearrange("p (h q) -> p h q", h=2)
                if m >= 0:
                    nc.vector.tensor_add(
                        ps3[:, :, 128 * m : 128 * (m + 1)],
                        ps3[:, :, 128 * m : 128 * (m + 1)],
                        mask_sb[:].rearrange("p (o q) -> p o q", o=1)
                        .to_broadcast((128, 2, 128)),
                    )
                if off == 0:
                    nc.scalar.activation(at[:], ps[:], AF.Exp)
                else:
                    nc.scalar.activation(
                        at3[:, :, off:512], ps3[:, :, off:512], AF.Exp)
                pending.append((j, off, at))
                if len(pending) > PEND:
                    emit_pv(hp, *pending.pop(0))
                it += 1
                while emitted < nfill and emitted * iters < it * nfill:
                    fillers[emitted]()
                    emitted += 1
            if c == NCH - 1 and hp == 1:
                for p in pending:
                    emit_pv(hp, *p)
                    self.rollout_qtile(c, p[0] - 4 * c, po_t[1], env)
            else:
                for p in pending:
                    emit_pv(hp, *p)
                self.rollout_hp(c, hp, po_t[hp], env)
                if c == NCH - 1:
                    # hp0 normalize for the last chunk, resolved during hp1
                    r2 = self.r2_units(c, env)
                    r2[0](); r2[1]()
        while emitted < nfill:
            fillers[emitted]()
            emitted += 1

    def rollout_hp(self, c, hp, po, env):
        """Copy unnormalized attn output to pairt (freeing po) and compute
        1/denominator: ACT Copy (no table switch) -> DMA reshape to [128,8]
        -> wide DVE reciprocal -> DMA back to a row for broadcasting."""
        nc = self.nc
        pairt = env["pairt"]
        cs = slice(512 * c, 512 * (c + 1))
        for h in range(2):
            nc.vector.tensor_copy(
                pairt[hp][64 * h : 64 * (h + 1), cs], po[0:64, h, :])
        denrow = env["small_p"].tile([1, 1024], dt.float32, tag=f"recf{hp}",
                                     name="denrow")
        nc.scalar.activation(denrow[0:1, :], po[64:65, :, :], AF.Copy)
        dencol = env["small_p"].tile([128, 8], dt.float32, tag=f"denc{hp}",
                                     name="dencol")
        nc.sync.dma_start(dencol[:], denrow[0:1, :])
        reccol = env["small_p"].tile([128, 8], dt.float32, tag=f"recc{hp}",
                                     name="reccol")
        nc.vector.reciprocal(reccol[:], dencol[:])
        recrow = env["small_p"].tile([1, 1024], dt.float32, tag=f"recb{hp}",
                                     name="recrow")
        nc.sync.dma_start(recrow[0:1, :], reccol[:])
        env["recip"][(c, hp)] = recrow

    def rollout_qtile(self, c, m, po, env):
        """Last-chunk hp1: normalize one query tile and immediately run its
        output projection + y store, pipelined against the remaining PVs."""
        nc = self.nc
        pairt, ps_p, y_p, y_d = env["pairt"], env["ps_p"], env["y_p"], env["y_d"]
        wo_sb = env["wo_sb"]
        t = 4 * c + m
        qs = slice(128 * t, 128 * (t + 1))
        ms = slice(128 * m, 128 * (m + 1))
        for h in range(2):
            nc.vector.tensor_copy(
                pairt[1][64 * h : 64 * (h + 1), qs], po[0:64, h, ms])
        sp = env["small_p"]
        denrow = sp.tile([1, 256], dt.float32, tag=f"qden{m}", name="denrow")
        nc.scalar.activation(denrow[0:1, :], po[64:65, :, ms], AF.Copy)
        dencol = sp.tile([32, 8], dt.float32, tag=f"qdenc{m}", name="dencol")
        nc.sync.dma_start(dencol[:], denrow[0:1, :])
        reccol = sp.tile([32, 8], dt.float32, tag=f"qrecc{m}", name="reccol")
        nc.vector.reciprocal(reccol[:], dencol[:])
        recrow = sp.tile([1, 256], dt.float32, tag=f"qrecb{m}", name="recrow")
        nc.sync.dma_start(recrow[0:1, :], reccol[:])
        bcsq = sp.tile([128, 128], dt.float32, tag=f"qbcs{m}", name="bcsq")
        for h in range(2):
            nc.sync.dma_start(
                bcsq[64 * h : 64 * (h + 1), :],
                recrow[0:1, 128 * h : 128 * (h + 1)]
                .rearrange("a (o n) -> a o n", o=1)
                .to_broadcast((1, 64, 128)),
            )
        for h in range(2):
            sl = pairt[1][64 * h : 64 * (h + 1), qs]
            eng = nc.vector if h == 0 else nc.gpsimd
            eng.tensor_mul(sl, sl, bcsq[64 * h : 64 * (h + 1), :])
        py = ps_p.tile([128, 1024], dt.float32, tag="ps", name="py")
        for o in range(2):
            for kt in range(2):
                nc.tensor.matmul(
                    py[:, 512 * o : 512 * (o + 1)],
                    pairt[kt][:, qs],
                    wo_sb[:, kt, 512 * o : 512 * (o + 1)],
                    start=(kt == 0), stop=(kt == 1),
                )
        ysb = y_p.tile([128, E], dt.bfloat16, tag="y", name="ysb")
        nc.vector.tensor_copy(ysb[:], py[:])
        nc.sync.dma_start(y_d[qs, :], ysb[:])
        if self.dbg:
            den_sb = env["small_p"].tile([1, 2048], dt.float32, tag="dens", name="dens")
            nc.vector.tensor_copy(den_sb[0:1, :], po[64:65, :, :])
            nc.sync.dma_start(self.dbg["den"][c], den_sb[0:1, :].rearrange("a n -> (a n)"))
            nc.sync.dma_start(self.dbg["recip"][c], recip_bf[0:1, :].rearrange("a n -> (a n)"))


# ----------------------------------------------------------------------
_PROGRAM = None


def _get_program():
    global _PROGRAM
    if _PROGRAM is None:
        _PROGRAM = _build_program()
    return _PROGRAM


def _make_in_maps(inputs, W_in, b_in, W_out, b_out):
    in_maps = []
    bf16 = ml_dtypes.bfloat16
    scale = 1.0 / np.sqrt(np.float32(HD))
    kr = np.arange(128)[:, None]
    qc = np.arange(128)[None, :]
    trimask = np.where(qc >= kr, 0.0, -1e30).astype(np.float32)
    for core in range(NC):
        b, g = divmod(core, 4)
        r = slice(256 * g, 256 * (g + 1))
        wq = W_in[0:E][r] * scale
        wk = W_in[E : 2 * E][r]
        wv = W_in[2 * E : 3 * E][r]
        xT = np.ascontiguousarray(inputs[b].T).astype(bf16)
        wqkT = np.ascontiguousarray(np.concatenate([wq, wk], axis=0).T).astype(bf16)
        wvT = np.ascontiguousarray(wv.T).astype(bf16)
        bq = (b_in[0:E][r] * scale).astype(np.float32)
        wo = np.ascontiguousarray(W_out[:, r].T).astype(bf16)
        in_maps.append(
            {
                "xT": xT,
                "wqkT": wqkT,
                "wvT": wvT,
                "bq": bq,
                "wo": wo,
                "trimask": trimask,
            }
        )
    return in_maps


def run_spmd(inputs, W_in, b_in, W_out, b_out, trace=False, **kw):
    nc = _get_program()
    in_maps = _make_in_maps(inputs, W_in, b_in, W_out, b_out)
    bkr = run_bass_kernel_spmd(nc, in_maps, list(range(NC)), trace=trace, **kw)
    parts = [bkr.results[i]["y"].astype(np.float32) for i in range(NC)]
    out = np.stack(
        [
            parts[0] + parts[1] + parts[2] + parts[3],
            parts[4] + parts[5] + parts[6] + parts[7],
        ]
    )
    yb = W_out.astype(np.float32) @ b_in[2 * E : 3 * E].astype(np.float32)
    out = out + (yb + b_out)[None, None, :]
    return out.astype(np.float32), bkr


def kernel(inputs, W_in, b_in, W_out, b_out):
    out, _ = run_spmd(
        np.asarray(inputs, dtype=np.float32),
        np.asarray(W_in, dtype=np.float32),
        np.asarray(b_in, dtype=np.float32),
        np.asarray(W_out, dtype=np.float32),
        np.asarray(b_out, dtype=np.float32),
    )
    return out


# revision 25
# speedup vs baseline: 1.3848x; 1.1511x over previous
"""Trainium2 Bass kernel for causal multi-head attention (B=2, S=2048, E=1024, H=16).

Sharding: 8 cores = 2 batches x 4 head-groups (4 heads each).
Each core computes its batch's QKV for its 4 heads, causal attention, and a
partial output projection; host sums the 4 group partials per batch, then adds
b_out and the (softmax-invariant-factored) W_out @ b_v term.

All matmul operands are bf16 (same PE rate as fp32r at 1 cyc/row, but valid at
any moving size, FWL weight loads, and half the DVE/DMA traffic).  PSUM stays
fp32.  Score matmuls have K=64 so the two heads of a pair run concurrently in
distinct PE row-groups.  k-bias is dropped (softmax-invariant), v-bias folded
into the host-side output add.
"""
import sys

sys.path.insert(0, "/opt/trn_rl_repo")

from contextlib import ExitStack

import ml_dtypes
import numpy as np

import concourse.bass as bass
import concourse.tile as tile
from concourse import bacc, mybir
from concourse.bass_utils import run_bass_kernel_spmd

dt = mybir.dt
AF = mybir.ActivationFunctionType

B, S, E, H = 2, 2048, 1024, 16
HD = 64                     # head dim
HPC = 4                     # heads per core
NC = 8                      # cores
KE = E // 128               # 8 contraction k-tiles for projections
NT = S // 128               # 16 token tiles
NCH = S // 512              # 4 token chunks
PEND = 4                    # pv emission delay (iterations)



DEBUG_OUTS = False


def _build_program():
    nc = bacc.Bacc("TRN2", target_bir_lowering=False, debug=False, num_devices=NC)

    xT_d = nc.dram_tensor("xT", [E, S], dt.bfloat16, kind="ExternalInput")
    wqkT_d = nc.dram_tensor("wqkT", [E, 512], dt.bfloat16, kind="ExternalInput")
    wvT_d = nc.dram_tensor("wvT", [E, 256], dt.bfloat16, kind="ExternalInput")
    bq_d = nc.dram_tensor("bq", [256], dt.float32, kind="ExternalInput")
    wo_d = nc.dram_tensor("wo", [256, E], dt.bfloat16, kind="ExternalInput")
    mask_d = nc.dram_tensor("trimask", [128, 128], dt.float32, kind="ExternalInput")
    y_d = nc.dram_tensor("y", [S, E], dt.bfloat16, kind="ExternalOutput")

    dbg = {}
    if DEBUG_OUTS:
        dbg["qkt"] = nc.dram_tensor("dbg_qkt", [4, 128, S], dt.bfloat16, kind="ExternalOutput")
        dbg["pair"] = nc.dram_tensor("dbg_pair", [2, 128, S], dt.bfloat16, kind="ExternalOutput")
        dbg["den"] = nc.dram_tensor("dbg_den", [NCH, 2048], dt.float32, kind="ExternalOutput")
        dbg["recip"] = nc.dram_tensor("dbg_recip", [NCH, 2048], dt.bfloat16, kind="ExternalOutput")
        dbg["bcs"] = nc.dram_tensor("dbg_bcs", [NCH, 128, 1024], dt.bfloat16, kind="ExternalOutput")
        dbg["vones"] = nc.dram_tensor("dbg_vones", [NT, 128, 260], dt.bfloat16, kind="ExternalOutput")

    with TileKernel(nc) as tk:
        tk.dbg = dbg
        tk.build(xT_d, wqkT_d, wvT_d, bq_d, wo_d, mask_d, y_d)
    nc.compile()
    return nc


class TileKernel:
    def __init__(self, nc):
        self.nc = nc
        self.dbg = {}
        self.ctx = ExitStack()
        self.tc_cm = tile.TileContext(nc)

    def __enter__(self):
        self.tc = self.tc_cm.__enter__()
        return self

    def __exit__(self, *a):
        self.ctx.close()
        return self.tc_cm.__exit__(*a)

    def build(self, xT_d, wqkT_d, wvT_d, bq_d, wo_d, mask_d, y_d):
        nc, tc, ctx = self.nc, self.tc, self.ctx
        pool = lambda name, bufs, **kw: ctx.enter_context(
            tc.tile_pool(name=name, bufs=bufs, **kw)
        )

        const_p = pool("const", 1)
        xs_p = pool("xs", 1)
        qkt_p = pool("qkt", 1)
        vones_p = pool("vones", 1)
        attn_p = pool("attn", PEND + 2)
        pair_p = pool("pair", 1)
        small_p = pool("small", 2)
        y_p = pool("y", 3)
        ps_p = pool("ps", 4, space="PSUM")     # 4 x [128,512] = 4 banks
        po_p = pool("po", 1, space="PSUM")     # 2 x [65, 2, 512] = 4 banks

        # ---- small consts first (fast DMAs) ----
        mask_sb = const_p.tile([128, 128], dt.float32, tag="mask")
        nc.sync.dma_start(mask_sb[:], mask_d[:])
        bq_sb = const_p.tile([128, 2], dt.float32, tag="bq")
        nc.sync.dma_start(bq_sb[:], bq_d[:].rearrange("(f p) -> p f", p=128))

        # ---- weights + x, interleaved per-ke so chunk-0 QKV starts early ----
        wqk_sb = const_p.tile([128, KE, 512], dt.bfloat16, tag="wqk")
        xs = xs_p.tile([128, KE, S], dt.bfloat16, tag="xs", name="xs")
        for ke in range(KE):
            nc.sync.dma_start(
                wqk_sb[:, ke, :],
                wqkT_d[128 * ke : 128 * (ke + 1), :],
            )
            nc.sync.dma_start(
                xs[:, ke, 0:512],
                xT_d[128 * ke : 128 * (ke + 1), 0:512],
            )
        wv_sb = const_p.tile([128, KE, 256], dt.bfloat16, tag="wv")
        nc.sync.dma_start(
            wv_sb[:],
            wvT_d[:].rearrange("(ke p) f -> p ke f", p=128),
        )
        for c in range(1, NCH):
            if c == 2:
                wo_sb = const_p.tile([128, 2, E], dt.bfloat16, tag="wo")
                nc.sync.dma_start(
                    wo_sb[:],
                    wo_d[:].rearrange("(kt p) f -> p kt f", p=128),
                )
            cs = slice(512 * c, 512 * (c + 1))
            nc.sync.dma_start(
                xs[:, :, cs],
                xT_d[:, cs].rearrange("(ke p) f -> p ke f", p=128),
            )

        # ---- persistent activations ----
        # qkt tiles: 0: q heads 0,1 | 1: q heads 2,3 | 2: k heads 0,1 | 3: k heads 2,3
        qkt = [qkt_p.tile([128, S], dt.bfloat16, tag=f"qkt{f}", name=f"qkt{f}")
               for f in range(4)]
        # vones[t]: per head [v(64) | 1] -> [128, 4, 65]
        vones = [vones_p.tile([128, 4, 65], dt.bfloat16, tag=f"v{t}", name=f"v{t}")
                 for t in range(NT)]
        for t in range(NT):
            nc.vector.memset(vones[t][:, :, 64:65], 1.0)
        # pairt[kt]: normalized attn output, [2 heads x 64 dims, S]
        pairt = [pair_p.tile([128, S], dt.bfloat16, tag=f"pair{hp}", name=f"pair{hp}")
                 for hp in range(2)]

        env = dict(
            xs=xs, wqk_sb=wqk_sb, wv_sb=wv_sb, bq_sb=bq_sb, wo_sb=wo_sb,
            mask_sb=mask_sb, qkt=qkt, vones=vones, pairt=pairt,
            xs_p=xs_p, ps_p=ps_p, po_p=po_p, attn_p=attn_p, small_p=small_p,
            y_p=y_p, y_d=y_d, po={}, recip={},
        )

        # startup: chunk-0 qkv emitted directly
        for u in self.qkv_units(0, env):
            u()
        for c in range(NCH):
            fillers = []
            if c == 1:
                fillers += self.r2_units(0, env) + self.qkv_units(2, env)
            elif c == 2:
                fillers += self.r2_units(1, env) + self.oproj_units(0, env)
                fillers += self.qkv_units(3, env)
            elif c == 3:
                fillers += self.r2_units(2, env)
                fillers += self.oproj_units(1, env) + self.oproj_units(2, env)
            elif c == 0:
                fillers += self.qkv_units(1, env)
            self.attention_chunk(c, env, fillers)
        if self.dbg:
            for f in range(4):
                nc.sync.dma_start(self.dbg["qkt"][f], qkt[f][:])
            for hp in range(2):
                nc.sync.dma_start(self.dbg["pair"][hp], pairt[hp][:])
            for t in range(NT):
                nc.sync.dma_start(
                    self.dbg["vones"][t],
                    vones[t][:].rearrange("p g d -> p (g d)"),
                )

    # ------------------------------------------------------------------
    def qkv_units(self, c, env):
        nc = self.nc
        cs = slice(512 * c, 512 * (c + 1))
        xs, wqk_sb, wv_sb = env["xs"], env["wqk_sb"], env["wv_sb"]
        bq_sb, qkt, vones = env["bq_sb"], env["qkt"], env["vones"]
        ps_p = env["ps_p"]
        units = []

        def qk_unit(f):
            pq = ps_p.tile([128, 512], dt.float32, tag="ps", name="pq")
            for ke in range(KE):
                nc.tensor.matmul(
                    pq[:, 0:512],
                    wqk_sb[:, ke, 128 * f : 128 * (f + 1)],
                    xs[:, ke, cs],
                    start=(ke == 0), stop=(ke == KE - 1),
                )
            if f < 2:
                nc.vector.tensor_scalar_add(
                    qkt[f][:, cs], pq[:, 0:512], bq_sb[:, f : f + 1])
            else:
                nc.vector.tensor_copy(qkt[f][:, cs], pq[:, 0:512])

        def v_unit(t4):
            t = 4 * c + t4
            pv = ps_p.tile([128, 512], dt.float32, tag="ps", name="pv")
            for ke in range(KE):
                nc.tensor.matmul(
                    pv[:, 0:256],
                    xs[:, ke, 512 * c + 128 * t4 : 512 * c + 128 * (t4 + 1)],
                    wv_sb[:, ke, :],
                    start=(ke == 0), stop=(ke == KE - 1),
                )
            nc.vector.tensor_copy(
                vones[t][:, :, 0:64],
                pv[:, 0:256].rearrange("p (g d) -> p g d", d=64),
            )

        for f in range(4):
            units.append(lambda f=f: qk_unit(f))
        for t4 in range(4):
            units.append(lambda t4=t4: v_unit(t4))
        return units

    # ------------------------------------------------------------------
    def r2_units(self, c, env):
        """Broadcast reciprocal denominators and normalize pairt in place."""
        nc = self.nc
        pairt = env["pairt"]
        cs = slice(512 * c, 512 * (c + 1))
        bcs = {}

        def bc_unit(hp):
            recrow = env["recip"][(c, hp)]
            sb = env["small_p"].tile([128, 512], dt.float32, tag=f"bcs{hp}",
                                     name="bcs")
            for h in range(2):
                nc.sync.dma_start(
                    sb[64 * h : 64 * (h + 1), :],
                    recrow[0:1, 512 * h : 512 * (h + 1)]
                    .rearrange("a (o n) -> a o n", o=1)
                    .to_broadcast((1, 64, 512)),
                )
            bcs[hp] = sb

        def mult_unit(hp):
            bc = bcs[hp]
            for h in range(2):
                sl = pairt[hp][64 * h : 64 * (h + 1), cs]
                eng = nc.vector if h == 0 else nc.gpsimd
                eng.tensor_mul(sl, sl, bc[64 * h : 64 * (h + 1), :])

        return [lambda: bc_unit(0), lambda: mult_unit(0),
                lambda: bc_unit(1), lambda: mult_unit(1)]

    # ------------------------------------------------------------------
    def oproj_units(self, c, env):
        nc = self.nc
        pairt, wo_sb, ps_p, y_p, y_d = (
            env["pairt"], env["wo_sb"], env["ps_p"], env["y_p"], env["y_d"])
        units = []

        def unit(t4):
            t = 4 * c + t4
            ysb = y_p.tile([128, E], dt.bfloat16, tag="y", name="ysb")
            for o in range(2):
                py = ps_p.tile([128, 512], dt.float32, tag="ps", name="py")
                for kt in range(2):
                    nc.tensor.matmul(
                        py[:],
                        pairt[kt][:, 128 * t : 128 * (t + 1)],
                        wo_sb[:, kt, 512 * o : 512 * (o + 1)],
                        start=(kt == 0), stop=(kt == 1),
                    )
                nc.vector.tensor_copy(ysb[:, 512 * o : 512 * (o + 1)], py[:])
            nc.gpsimd.dma_start(y_d[128 * t : 128 * (t + 1), :], ysb[:])

        for t4 in range(4):
            units.append(lambda t4=t4: unit(t4))
        return units

    # ------------------------------------------------------------------
    def attention_chunk(self, c, env, fillers):
        """Attention for both head pairs of chunk c, weaving filler units
        (prev-chunk rollout/out-proj, next-chunk qkv) into the PE stream."""
        nc = self.nc
        qkt, vones, mask_sb = env["qkt"], env["vones"], env["mask_sb"]
        ps_p, po_p, attn_p = env["ps_p"], env["po_p"], env["attn_p"]
        nj = 4 * c + 4
        po_t = [po_p.tile([65, 2, 512], dt.float32, tag=f"po{hp}", name="po")
                for hp in range(2)]

        nfill = len(fillers)
        iters = 2 * nj
        emitted = 0

        def emit_pv(hp, j, off, at):
            for h in range(2):
                i = 2 * hp + h
                nc.tensor.matmul(
                    po_t[hp][:, h, off:512],
                    vones[j][:, i, :],
                    at[:, 512 * h + off : 512 * (h + 1)],
                    start=(j == 0), stop=(j == nj - 1),
                    skip_group_check=True,
                )

        it = 0
        for hp in range(2):
            pending = []
            for j in range(nj):
                at = attn_p.tile([128, 1024], dt.bfloat16, tag="attn", name="at")
                m = j - 4 * c
                off = 128 * m if m >= 1 else 0
                for h in range(2):
                    r0 = 64 * h
                    ps = ps_p.tile([128, 512], dt.float32, tag="ps", name="ps")
                    nc.tensor.matmul(
                        ps[:, off:512],
                        qkt[2 + hp][r0 : r0 + 64, 128 * j : 128 * (j + 1)],
                        qkt[hp][r0 : r0 + 64, 512 * c + off : 512 * (c + 1)],
                        start=True, stop=True,
                    )
                    if m >= 0:
                        nc.vector.tensor_add(
                            ps[:, 128 * m : 128 * (m + 1)],
                            ps[:, 128 * m : 128 * (m + 1)],
                            mask_sb[:],
                        )
                    nc.scalar.activation(
                        at[:, 512 * h + off : 512 * (h + 1)], ps[:, off:512],
                        AF.Exp)
                pending.append((j, off, at))
                if len(pending) > PEND:
                    emit_pv(hp, *pending.pop(0))
                it += 1
                while emitted < nfill and emitted * iters < it * nfill:
                    fillers[emitted]()
                    emitted += 1
            if c == NCH - 1 and hp == 1:
                for p in pending:
                    emit_pv(hp, *p)
                    self.rollout_qtile(c, p[0] - 4 * c, po_t[1], env)
            else:
                for p in pending:
                    emit_pv(hp, *p)
                self.rollout_hp(c, hp, po_t[hp], env)
                if c == NCH - 1:
                    # hp0 normalize for the last chunk, resolved during hp1
                    r2 = self.r2_units(c, env)
                    r2[0](); r2[1]()
        while emitted < nfill:
            fillers[emitted]()
            emitted += 1

    def rollout_hp(self, c, hp, po, env):
        """Copy unnormalized attn output to pairt (freeing po) and compute
        1/denominator: ACT Copy (no table switch) -> DMA reshape to [128,8]
        -> wide DVE reciprocal -> DMA back to a row for broadcasting."""
        nc = self.nc
        pairt = env["pairt"]
        cs = slice(512 * c, 512 * (c + 1))
        for h in range(2):
            nc.vector.tensor_copy(
                pairt[hp][64 * h : 64 * (h + 1), cs], po[0:64, h, :])
        denrow = env["small_p"].tile([1, 1024], dt.float32, tag=f"recf{hp}",
                                     name="denrow")
        nc.scalar.activation(denrow[0:1, :], po[64:65, :, :], AF.Copy)
        dencol = env["small_p"].tile([128, 8], dt.float32, tag=f"denc{hp}",
                                     name="dencol")
        nc.sync.dma_start(dencol[:], denrow[0:1, :])
        reccol = env["small_p"].tile([128, 8], dt.float32, tag=f"recc{hp}",
                                     name="reccol")
        nc.vector.reciprocal(reccol[:], dencol[:])
        recrow = env["small_p"].tile([1, 1024], dt.float32, tag=f"recb{hp}",
                                     name="recrow")
        nc.sync.dma_start(recrow[0:1, :], reccol[:])
        env["recip"][(c, hp)] = recrow

    def rollout_qtile(self, c, m, po, env):
        """Last-chunk hp1: normalize one query tile and immediately run its
        output projection + y store, pipelined against the remaining PVs."""
        nc = self.nc
        pairt, ps_p, y_p, y_d = env["pairt"], env["ps_p"], env["y_p"], env["y_d"]
        wo_sb = env["wo_sb"]
        t = 4 * c + m
        qs = slice(128 * t, 128 * (t + 1))
        ms = slice(128 * m, 128 * (m + 1))
        for h in range(2):
            nc.vector.tensor_copy(
                pairt[1][64 * h : 64 * (h + 1), qs], po[0:64, h, ms])
        sp = env["small_p"]
        denrow = sp.tile([1, 256], dt.float32, tag=f"qden{m}", name="denrow")
        nc.scalar.activation(denrow[0:1, :], po[64:65, :, ms], AF.Copy)
        dencol = sp.tile([32, 8], dt.float32, tag=f"qdenc{m}", name="dencol")
        nc.sync.dma_start(dencol[:], denrow[0:1, :])
        reccol = sp.tile([32, 8], dt.float32, tag=f"qrecc{m}", name="reccol")
        nc.vector.reciprocal(reccol[:], dencol[:])
        recrow = sp.tile([1, 256], dt.float32, tag=f"qrecb{m}", name="recrow")
        nc.sync.dma_start(recrow[0:1, :], reccol[:])
        bcsq = sp.tile([128, 128], dt.float32, tag=f"qbcs{m}", name="bcsq")
        for h in range(2):
            nc.sync.dma_start(
                bcsq[64 * h : 64 * (h + 1), :],
                recrow[0:1, 128 * h : 128 * (h + 1)]
                .rearrange("a (o n) -> a o n", o=1)
                .to_broadcast((1, 64, 128)),
            )
        for h in range(2):
            sl = pairt[1][64 * h : 64 * (h + 1), qs]
            eng = nc.vector if h == 0 else nc.gpsimd
            eng.tensor_mul(sl, sl, bcsq[64 * h : 64 * (h + 1), :])
        ysb = y_p.tile([128, E], dt.bfloat16, tag="y", name="ysb")
        for o in range(2):
            py = ps_p.tile([128, 512], dt.float32, tag="ps", name="py")
            for kt in range(2):
                nc.tensor.matmul(
                    py[:],
                    pairt[kt][:, qs],
                    wo_sb[:, kt, 512 * o : 512 * (o + 1)],
                    start=(kt == 0), stop=(kt == 1),
                )
            nc.vector.tensor_copy(ysb[:, 512 * o : 512 * (o + 1)], py[:])
        nc.scalar.dma_start(y_d[qs, :], ysb[:])
        if self.dbg:
            den_sb = env["small_p"].tile([1, 2048], dt.float32, tag="dens", name="dens")
            nc.vector.tensor_copy(den_sb[0:1, :], po[64:65, :, :])
            nc.sync.dma_start(self.dbg["den"][c], den_sb[0:1, :].rearrange("a n -> (a n)"))
            nc.sync.dma_start(self.dbg["recip"][c], recip_bf[0:1, :].rearrange("a n -> (a n)"))


# ----------------------------------------------------------------------
_PROGRAM = None


def _get_program():
    global _PROGRAM
    if _PROGRAM is None:
        _PROGRAM = _build_program()
    return _PROGRAM


def _make_in_maps(inputs, W_in, b_in, W_out, b_out):
    in_maps = []
    bf16 = ml_dtypes.bfloat16
    scale = 1.0 / np.sqrt(np.float32(HD))
    kr = np.arange(128)[:, None]
    qc = np.arange(128)[None, :]
    trimask = np.where(qc >= kr, 0.0, -1e30).astype(np.float32)
    for core in range(NC):
        b, g = divmod(core, 4)
        r = slice(256 * g, 256 * (g + 1))
        wq = W_in[0:E][r] * scale
        wk = W_in[E : 2 * E][r]
        wv = W_in[2 * E : 3 * E][r]
        xT = np.ascontiguousarray(inputs[b].T).astype(bf16)
        wqkT = np.ascontiguousarray(np.concatenate([wq, wk], axis=0).T).astype(bf16)
        wvT = np.ascontiguousarray(wv.T).astype(bf16)
        bq = (b_in[0:E][r] * scale).astype(np.float32)
        wo = np.ascontiguousarray(W_out[:, r].T).astype(bf16)
        in_maps.append(
            {
                "xT": xT,
                "wqkT": wqkT,
                "wvT": wvT,
                "bq": bq,
                "wo": wo,
                "trimask": trimask,
            }
        )
    return in_maps


def run_spmd(inputs, W_in, b_in, W_out, b_out, trace=False, **kw):
    nc = _get_program()
    in_maps = _make_in_maps(inputs, W_in, b_in, W_out, b_out)
    bkr = run_bass_kernel_spmd(nc, in_maps, list(range(NC)), trace=trace, **kw)
    parts = [bkr.results[i]["y"].astype(np.float32) for i in range(NC)]
    out = np.stack(
        [
            parts[0] + parts[1] + parts[2] + parts[3],
            parts[4] + parts[5] + parts[6] + parts[7],
        ]
    )
    yb = W_out.astype(np.float32) @ b_in[2 * E : 3 * E].astype(np.float32)
    out = out + (yb + b_out)[None, None, :]
    return out.astype(np.float32), bkr


def kernel(inputs, W_in, b_in, W_out, b_out):
    out, _ = run_spmd(
        np.asarray(inputs, dtype=np.float32),
        np.asarray(W_in, dtype=np.float32),
        np.asarray(b_in, dtype=np.float32),
        np.asarray(W_out, dtype=np.float32),
        np.asarray(b_out, dtype=np.float32),
    )
    return out


# revision 26
# speedup vs baseline: 1.4344x; 1.0358x over previous
"""Trainium2 Bass kernel for causal multi-head attention (B=2, S=2048, E=1024, H=16).

Sharding: 8 cores = 2 batches x 4 head-groups (4 heads each).
Each core computes its batch's QKV for its 4 heads, causal attention, and a
partial output projection; host sums the 4 group partials per batch, then adds
b_out and the (softmax-invariant-factored) W_out @ b_v term.

All matmul operands are bf16 (same PE rate as fp32r at 1 cyc/row, but valid at
any moving size, FWL weight loads, and half the DVE/DMA traffic).  PSUM stays
fp32.  Score matmuls have K=64 so the two heads of a pair run concurrently in
distinct PE row-groups.  k-bias is dropped (softmax-invariant), v-bias folded
into the host-side output add.
"""
import sys

sys.path.insert(0, "/opt/trn_rl_repo")

from contextlib import ExitStack

import ml_dtypes
import numpy as np

import concourse.bass as bass
import concourse.tile as tile
from concourse import bacc, mybir
from concourse.bass_utils import run_bass_kernel_spmd

dt = mybir.dt
AF = mybir.ActivationFunctionType

B, S, E, H = 2, 2048, 1024, 16
HD = 64                     # head dim
HPC = 4                     # heads per core
NC = 8                      # cores
KE = E // 128               # 8 contraction k-tiles for projections
NT = S // 128               # 16 token tiles
NCH = S // 512              # 4 token chunks
PEND = 4                    # pv emission delay (iterations)



DEBUG_OUTS = False


def _build_program():
    nc = bacc.Bacc("TRN2", target_bir_lowering=False, debug=False, num_devices=NC)

    xT_d = nc.dram_tensor("xT", [E, S], dt.bfloat16, kind="ExternalInput")
    wqkT_d = nc.dram_tensor("wqkT", [E, 512], dt.bfloat16, kind="ExternalInput")
    wvT_d = nc.dram_tensor("wvT", [E, 256], dt.bfloat16, kind="ExternalInput")
    bq_d = nc.dram_tensor("bq", [256], dt.float32, kind="ExternalInput")
    wo_d = nc.dram_tensor("wo", [256, E], dt.bfloat16, kind="ExternalInput")
    mask_d = nc.dram_tensor("trimask", [128, 128], dt.float32, kind="ExternalInput")
    y_d = nc.dram_tensor("y", [S, E], dt.bfloat16, kind="ExternalOutput")

    dbg = {}
    if DEBUG_OUTS:
        dbg["qkt"] = nc.dram_tensor("dbg_qkt", [4, 128, S], dt.bfloat16, kind="ExternalOutput")
        dbg["pair"] = nc.dram_tensor("dbg_pair", [2, 128, S], dt.bfloat16, kind="ExternalOutput")
        dbg["den"] = nc.dram_tensor("dbg_den", [NCH, 2048], dt.float32, kind="ExternalOutput")
        dbg["recip"] = nc.dram_tensor("dbg_recip", [NCH, 2048], dt.bfloat16, kind="ExternalOutput")
        dbg["bcs"] = nc.dram_tensor("dbg_bcs", [NCH, 128, 1024], dt.bfloat16, kind="ExternalOutput")
        dbg["vones"] = nc.dram_tensor("dbg_vones", [NT, 128, 260], dt.bfloat16, kind="ExternalOutput")

    with TileKernel(nc) as tk:
        tk.dbg = dbg
        tk.build(xT_d, wqkT_d, wvT_d, bq_d, wo_d, mask_d, y_d)
    nc.compile()
    return nc


class TileKernel:
    def __init__(self, nc):
        self.nc = nc
        self.dbg = {}
        self.ctx = ExitStack()
        self.tc_cm = tile.TileContext(nc)

    def __enter__(self):
        self.tc = self.tc_cm.__enter__()
        return self

    def __exit__(self, *a):
        self.ctx.close()
        return self.tc_cm.__exit__(*a)

    def build(self, xT_d, wqkT_d, wvT_d, bq_d, wo_d, mask_d, y_d):
        nc, tc, ctx = self.nc, self.tc, self.ctx
        pool = lambda name, bufs, **kw: ctx.enter_context(
            tc.tile_pool(name=name, bufs=bufs, **kw)
        )

        const_p = pool("const", 1)
        xs_p = pool("xs", 1)
        qkt_p = pool("qkt", 1)
        vones_p = pool("vones", 1)
        attn_p = pool("attn", PEND + 2)
        pair_p = pool("pair", 1)
        small_p = pool("small", 2)
        y_p = pool("y", 3)
        ps_p = pool("ps", 3, space="PSUM")     # 3 x [128,1024] = 6 banks
        po_p = pool("po", 1, space="PSUM")     # [65, 2, 512] shared = 2 banks

        # ---- small consts first (fast DMAs) ----
        mask_sb = const_p.tile([128, 128], dt.float32, tag="mask")
        nc.sync.dma_start(mask_sb[:], mask_d[:])
        bq_sb = const_p.tile([128, 2], dt.float32, tag="bq")
        nc.sync.dma_start(bq_sb[:], bq_d[:].rearrange("(f p) -> p f", p=128))

        # ---- weights + x, interleaved per-ke so chunk-0 QKV starts early ----
        wqk_sb = const_p.tile([128, KE, 512], dt.bfloat16, tag="wqk")
        xs = xs_p.tile([128, KE, S], dt.bfloat16, tag="xs", name="xs")
        for ke in range(KE):
            nc.sync.dma_start(
                wqk_sb[:, ke, :],
                wqkT_d[128 * ke : 128 * (ke + 1), :],
            )
            nc.sync.dma_start(
                xs[:, ke, 0:512],
                xT_d[128 * ke : 128 * (ke + 1), 0:512],
            )
        wv_sb = const_p.tile([128, KE, 256], dt.bfloat16, tag="wv")
        nc.sync.dma_start(
            wv_sb[:],
            wvT_d[:].rearrange("(ke p) f -> p ke f", p=128),
        )
        for c in range(1, NCH):
            if c == 2:
                wo_sb = const_p.tile([128, 2, E], dt.bfloat16, tag="wo")
                nc.sync.dma_start(
                    wo_sb[:],
                    wo_d[:].rearrange("(kt p) f -> p kt f", p=128),
                )
            cs = slice(512 * c, 512 * (c + 1))
            nc.sync.dma_start(
                xs[:, :, cs],
                xT_d[:, cs].rearrange("(ke p) f -> p ke f", p=128),
            )

        # ---- persistent activations ----
        # qkt tiles: 0: q heads 0,1 | 1: q heads 2,3 | 2: k heads 0,1 | 3: k heads 2,3
        qkt = [qkt_p.tile([128, S], dt.bfloat16, tag=f"qkt{f}", name=f"qkt{f}")
               for f in range(4)]
        # vones[t]: per head [v(64) | 1] -> [128, 4, 65]
        vones = [vones_p.tile([128, 4, 65], dt.bfloat16, tag=f"v{t}", name=f"v{t}")
                 for t in range(NT)]
        for t in range(NT):
            nc.vector.memset(vones[t][:, :, 64:65], 1.0)
        # pairt[kt]: normalized attn output, [2 heads x 64 dims, S]
        pairt = [pair_p.tile([128, S], dt.bfloat16, tag=f"pair{hp}", name=f"pair{hp}")
                 for hp in range(2)]

        env = dict(
            xs=xs, wqk_sb=wqk_sb, wv_sb=wv_sb, bq_sb=bq_sb, wo_sb=wo_sb,
            mask_sb=mask_sb, qkt=qkt, vones=vones, pairt=pairt,
            xs_p=xs_p, ps_p=ps_p, po_p=po_p, attn_p=attn_p, small_p=small_p,
            y_p=y_p, y_d=y_d, po={}, recip={},
        )

        # startup: chunk-0 qkv emitted directly
        for u in self.qkv_units(0, env):
            u()
        for c in range(NCH):
            fillers = []
            if c == 1:
                fillers += self.r2_units(0, env) + self.qkv_units(2, env)
            elif c == 2:
                fillers += self.r2_units(1, env) + self.oproj_units(0, env)
                fillers += self.qkv_units(3, env)
            elif c == 3:
                fillers += self.r2_units(2, env)
                fillers += self.oproj_units(1, env) + self.oproj_units(2, env)
            elif c == 0:
                fillers += self.qkv_units(1, env)
            self.attention_chunk(c, env, fillers)
        if self.dbg:
            for f in range(4):
                nc.sync.dma_start(self.dbg["qkt"][f], qkt[f][:])
            for hp in range(2):
                nc.sync.dma_start(self.dbg["pair"][hp], pairt[hp][:])
            for t in range(NT):
                nc.sync.dma_start(
                    self.dbg["vones"][t],
                    vones[t][:].rearrange("p g d -> p (g d)"),
                )

    # ------------------------------------------------------------------
    def qkv_units(self, c, env):
        nc = self.nc
        cs = slice(512 * c, 512 * (c + 1))
        xs, wqk_sb, wv_sb = env["xs"], env["wqk_sb"], env["wv_sb"]
        bq_sb, qkt, vones = env["bq_sb"], env["qkt"], env["vones"]
        ps_p = env["ps_p"]
        units = []

        def qk_unit(f):
            pq = ps_p.tile([128, 1024], dt.float32, tag="ps", name="pq")
            for ke in range(KE):
                nc.tensor.matmul(
                    pq[:, 0:512],
                    wqk_sb[:, ke, 128 * f : 128 * (f + 1)],
                    xs[:, ke, cs],
                    start=(ke == 0), stop=(ke == KE - 1),
                )
            if f < 2:
                nc.vector.tensor_scalar_add(
                    qkt[f][:, cs], pq[:, 0:512], bq_sb[:, f : f + 1])
            else:
                nc.vector.tensor_copy(qkt[f][:, cs], pq[:, 0:512])

        def v_unit(t4):
            t = 4 * c + t4
            pv = ps_p.tile([128, 1024], dt.float32, tag="ps", name="pv")
            for ke in range(KE):
                nc.tensor.matmul(
                    pv[:, 0:256],
                    xs[:, ke, 512 * c + 128 * t4 : 512 * c + 128 * (t4 + 1)],
                    wv_sb[:, ke, :],
                    start=(ke == 0), stop=(ke == KE - 1),
                )
            nc.vector.tensor_copy(
                vones[t][:, :, 0:64],
                pv[:, 0:256].rearrange("p (g d) -> p g d", d=64),
            )

        for f in range(4):
            units.append(lambda f=f: qk_unit(f))
        for t4 in range(4):
            units.append(lambda t4=t4: v_unit(t4))
        return units

    # ------------------------------------------------------------------
    def r2_units(self, c, env):
        """Broadcast reciprocal denominators and normalize pairt in place."""
        nc = self.nc
        pairt = env["pairt"]
        cs = slice(512 * c, 512 * (c + 1))
        bcs = {}

        def bc_unit(hp):
            recrow = env["recip"][(c, hp)]
            sb = env["small_p"].tile([128, 512], dt.float32, tag=f"bcs{hp}",
                                     name="bcs")
            for h in range(2):
                nc.sync.dma_start(
                    sb[64 * h : 64 * (h + 1), :],
                    recrow[0:1, 512 * h : 512 * (h + 1)]
                    .rearrange("a (o n) -> a o n", o=1)
                    .to_broadcast((1, 64, 512)),
                )
            bcs[hp] = sb

        def mult_unit(hp):
            bc = bcs[hp]
            for h in range(2):
                sl = pairt[hp][64 * h : 64 * (h + 1), cs]
                eng = nc.vector if h == 0 else nc.gpsimd
                eng.tensor_mul(sl, sl, bc[64 * h : 64 * (h + 1), :])

        return [lambda: bc_unit(0), lambda: mult_unit(0),
                lambda: bc_unit(1), lambda: mult_unit(1)]

    # ------------------------------------------------------------------
    def oproj_units(self, c, env):
        nc = self.nc
        pairt, wo_sb, ps_p, y_p, y_d = (
            env["pairt"], env["wo_sb"], env["ps_p"], env["y_p"], env["y_d"])
        units = []

        def unit(t4):
            t = 4 * c + t4
            ysb = y_p.tile([128, E], dt.bfloat16, tag="y", name="ysb")
            py = ps_p.tile([128, 1024], dt.float32, tag="ps", name="py")
            for o in range(2):
                for kt in range(2):
                    nc.tensor.matmul(
                        py[:, 512 * o : 512 * (o + 1)],
                        pairt[kt][:, 128 * t : 128 * (t + 1)],
                        wo_sb[:, kt, 512 * o : 512 * (o + 1)],
                        start=(kt == 0), stop=(kt == 1),
                    )
            nc.vector.tensor_copy(ysb[:], py[:])
            nc.gpsimd.dma_start(y_d[128 * t : 128 * (t + 1), :], ysb[:])

        for t4 in range(4):
            units.append(lambda t4=t4: unit(t4))
        return units

    # ------------------------------------------------------------------
    def attention_chunk(self, c, env, fillers):
        """Attention for both head pairs of chunk c, weaving filler units
        (prev-chunk rollout/out-proj, next-chunk qkv) into the PE stream."""
        nc = self.nc
        qkt, vones, mask_sb = env["qkt"], env["vones"], env["mask_sb"]
        ps_p, po_p, attn_p = env["ps_p"], env["po_p"], env["attn_p"]
        nj = 4 * c + 4

        nfill = len(fillers)
        iters = 2 * nj
        emitted = 0

        def emit_pv(hp, j, off, at):
            for h in range(2):
                i = 2 * hp + h
                nc.tensor.matmul(
                    po_t[hp][:, h, off:512],
                    vones[j][:, i, :],
                    at[:, 512 * h + off : 512 * (h + 1)],
                    start=(j == 0), stop=(j == nj - 1),
                    skip_group_check=True,
                )

        it = 0
        po_t = [None, None]
        for hp in range(2):
            po_t[hp] = po_p.tile([65, 2, 512], dt.float32, tag="po", name="po")
            pending = []
            for j in range(nj):
                at = attn_p.tile([128, 1024], dt.bfloat16, tag="attn", name="at")
                ps = ps_p.tile([128, 1024], dt.float32, tag="ps", name="ps")
                m = j - 4 * c
                off = 128 * m if m >= 1 else 0
                for h in range(2):
                    r0 = 64 * h
                    nc.tensor.matmul(
                        ps[:, 512 * h + off : 512 * (h + 1)],
                        qkt[2 + hp][r0 : r0 + 64, 128 * j : 128 * (j + 1)],
                        qkt[hp][r0 : r0 + 64, 512 * c + off : 512 * (c + 1)],
                        start=True, stop=True,
                    )
                    if m >= 0:
                        lo = 512 * h + 128 * m
                        nc.vector.tensor_add(
                            ps[:, lo : lo + 128], ps[:, lo : lo + 128],
                            mask_sb[:],
                        )
                if off == 0:
                    nc.scalar.activation(at[:], ps[:], AF.Exp)
                else:
                    for h in range(2):
                        nc.scalar.activation(
                            at[:, 512 * h + off : 512 * (h + 1)],
                            ps[:, 512 * h + off : 512 * (h + 1)], AF.Exp)
                pending.append((j, off, at))
                if len(pending) > PEND:
                    emit_pv(hp, *pending.pop(0))
                it += 1
                while emitted < nfill and emitted * iters < it * nfill:
                    fillers[emitted]()
                    emitted += 1
            if c == NCH - 1 and hp == 1:
                for p in pending:
                    emit_pv(hp, *p)
                    self.rollout_qtile(c, p[0] - 4 * c, po_t[1], env)
            else:
                for p in pending:
                    emit_pv(hp, *p)
                self.rollout_hp(c, hp, po_t[hp], env)
                if c == NCH - 1:
                    # hp0 normalize for the last chunk, resolved during hp1
                    r2 = self.r2_units(c, env)
                    r2[0](); r2[1]()
        while emitted < nfill:
            fillers[emitted]()
            emitted += 1

    def rollout_hp(self, c, hp, po, env):
        """Copy unnormalized attn output to pairt (freeing po) and compute
        1/denominator: ACT Copy (no table switch) -> DMA reshape to [128,8]
        -> wide DVE reciprocal -> DMA back to a row for broadcasting."""
        nc = self.nc
        pairt = env["pairt"]
        cs = slice(512 * c, 512 * (c + 1))
        for h in range(2):
            nc.vector.tensor_copy(
                pairt[hp][64 * h : 64 * (h + 1), cs], po[0:64, h, :])
        denrow = env["small_p"].tile([1, 1024], dt.float32, tag=f"recf{hp}",
                                     name="denrow")
        nc.scalar.activation(denrow[0:1, :], po[64:65, :, :], AF.Copy)
        dencol = env["small_p"].tile([128, 8], dt.float32, tag=f"denc{hp}",
                                     name="dencol")
        nc.sync.dma_start(dencol[:], denrow[0:1, :])
        reccol = env["small_p"].tile([128, 8], dt.float32, tag=f"recc{hp}",
                                     name="reccol")
        nc.vector.reciprocal(reccol[:], dencol[:])
        recrow = env["small_p"].tile([1, 1024], dt.float32, tag=f"recb{hp}",
                                     name="recrow")
        nc.sync.dma_start(recrow[0:1, :], reccol[:])
        env["recip"][(c, hp)] = recrow

    def rollout_qtile(self, c, m, po, env):
        """Last-chunk hp1: normalize one query tile and immediately run its
        output projection + y store, pipelined against the remaining PVs."""
        nc = self.nc
        pairt, ps_p, y_p, y_d = env["pairt"], env["ps_p"], env["y_p"], env["y_d"]
        wo_sb = env["wo_sb"]
        t = 4 * c + m
        qs = slice(128 * t, 128 * (t + 1))
        ms = slice(128 * m, 128 * (m + 1))
        for h in range(2):
            nc.vector.tensor_copy(
                pairt[1][64 * h : 64 * (h + 1), qs], po[0:64, h, ms])
        sp = env["small_p"]
        denrow = sp.tile([1, 256], dt.float32, tag=f"qden{m}", name="denrow")
        nc.scalar.activation(denrow[0:1, :], po[64:65, :, ms], AF.Copy)
        dencol = sp.tile([32, 8], dt.float32, tag=f"qdenc{m}", name="dencol")
        nc.sync.dma_start(dencol[:], denrow[0:1, :])
        reccol = sp.tile([32, 8], dt.float32, tag=f"qrecc{m}", name="reccol")
        nc.vector.reciprocal(reccol[:], dencol[:])
        recrow = sp.tile([1, 256], dt.float32, tag=f"qrecb{m}", name="recrow")
        nc.sync.dma_start(recrow[0:1, :], reccol[:])
        bcsq = sp.tile([128, 128], dt.float32, tag=f"qbcs{m}", name="bcsq")
        for h in range(2):
            nc.sync.dma_start(
                bcsq[64 * h : 64 * (h + 1), :],
                recrow[0:1, 128 * h : 128 * (h + 1)]
                .rearrange("a (o n) -> a o n", o=1)
                .to_broadcast((1, 64, 128)),
            )
        for h in range(2):
            sl = pairt[1][64 * h : 64 * (h + 1), qs]
            eng = nc.vector if h == 0 else nc.gpsimd
            eng.tensor_mul(sl, sl, bcsq[64 * h : 64 * (h + 1), :])
        ysb = y_p.tile([128, E], dt.bfloat16, tag="y", name="ysb")
        py = ps_p.tile([128, 1024], dt.float32, tag="ps", name="py")
        for o in range(2):
            for kt in range(2):
                nc.tensor.matmul(
                    py[:, 512 * o : 512 * (o + 1)],
                    pairt[kt][:, qs],
                    wo_sb[:, kt, 512 * o : 512 * (o + 1)],
                    start=(kt == 0), stop=(kt == 1),
                )
        nc.vector.tensor_copy(ysb[:], py[:])
        nc.scalar.dma_start(y_d[qs, :], ysb[:])
        if self.dbg:
            den_sb = env["small_p"].tile([1, 2048], dt.float32, tag="dens", name="dens")
            nc.vector.tensor_copy(den_sb[0:1, :], po[64:65, :, :])
            nc.sync.dma_start(self.dbg["den"][c], den_sb[0:1, :].rearrange("a n -> (a n)"))
            nc.sync.dma_start(self.dbg["recip"][c], recip_bf[0:1, :].rearrange("a n -> (a n)"))


# ----------------------------------------------------------------------
_PROGRAM = None


def _get_program():
    global _PROGRAM
    if _PROGRAM is None:
        _PROGRAM = _build_program()
    return _PROGRAM


def _make_in_maps(inputs, W_in, b_in, W_out, b_out):
    in_maps = []
    bf16 = ml_dtypes.bfloat16
    scale = 1.0 / np.sqrt(np.float32(HD))
    kr = np.arange(128)[:, None]
    qc = np.arange(128)[None, :]
    trimask = np.where(qc >= kr, 0.0, -1e30).astype(np.float32)
    for core in range(NC):
        b, g = divmod(core, 4)
        r = slice(256 * g, 256 * (g + 1))
        wq = W_in[0:E][r] * scale
        wk = W_in[E : 2 * E][r]
        wv = W_in[2 * E : 3 * E][r]
        xT = np.ascontiguousarray(inputs[b].T).astype(bf16)
        wqkT = np.ascontiguousarray(np.concatenate([wq, wk], axis=0).T).astype(bf16)
        wvT = np.ascontiguousarray(wv.T).astype(bf16)
        bq = (b_in[0:E][r] * scale).astype(np.float32)
        wo = np.ascontiguousarray(W_out[:, r].T).astype(bf16)
        in_maps.append(
            {
                "xT": xT,
                "wqkT": wqkT,
                "wvT": wvT,
                "bq": bq,
                "wo": wo,
                "trimask": trimask,
            }
        )
    return in_maps


def run_spmd(inputs, W_in, b_in, W_out, b_out, trace=False, **kw):
    nc = _get_program()
    in_maps = _make_in_maps(inputs, W_in, b_in, W_out, b_out)
    bkr = run_bass_kernel_spmd(nc, in_maps, list(range(NC)), trace=trace, **kw)
    parts = [bkr.results[i]["y"].astype(np.float32) for i in range(NC)]
    out = np.stack(
        [
            parts[0] + parts[1] + parts[2] + parts[3],
            parts[4] + parts[5] + parts[6] + parts[7],
        ]
    )
    yb = W_out.astype(np.float32) @ b_in[2 * E : 3 * E].astype(np.float32)
    out = out + (yb + b_out)[None, None, :]
    return out.astype(np.float32), bkr


def kernel(inputs, W_in, b_in, W_out, b_out):
    out, _ = run_spmd(
        np.asarray(inputs, dtype=np.float32),
        np.asarray(W_in, dtype=np.float32),
        np.asarray(b_in, dtype=np.float32),
        np.asarray(W_out, dtype=np.float32),
        np.asarray(b_out, dtype=np.float32),
    )
    return out


# revision 28
# speedup vs baseline: 1.4377x; 1.0023x over previous
"""Trainium2 Bass kernel for causal multi-head attention (B=2, S=2048, E=1024, H=16).

Sharding: 8 cores = 2 batches x 4 head-groups (4 heads each).
Each core computes its batch's QKV for its 4 heads, causal attention, and a
partial output projection; host sums the 4 group partials per batch, then adds
b_out and the (softmax-invariant-factored) W_out @ b_v term.

All matmul operands are bf16 (same PE rate as fp32r at 1 cyc/row, but valid at
any moving size, FWL weight loads, and half the DVE/DMA traffic).  PSUM stays
fp32.  Score matmuls have K=64 so the two heads of a pair run concurrently in
distinct PE row-groups.  k-bias is dropped (softmax-invariant), v-bias folded
into the host-side output add.
"""
import sys

sys.path.insert(0, "/opt/trn_rl_repo")

from contextlib import ExitStack

import ml_dtypes
import numpy as np

import concourse.bass as bass
import concourse.tile as tile
from concourse import bacc, mybir
from concourse.bass_utils import run_bass_kernel_spmd

dt = mybir.dt
AF = mybir.ActivationFunctionType

B, S, E, H = 2, 2048, 1024, 16
HD = 64                     # head dim
HPC = 4                     # heads per core
NC = 8                      # cores
KE = E // 128               # 8 contraction k-tiles for projections
NT = S // 128               # 16 token tiles
NCH = S // 512              # 4 token chunks
PEND = 4                    # pv emission delay (iterations)



DEBUG_OUTS = False


def _build_program():
    nc = bacc.Bacc("TRN2", target_bir_lowering=False, debug=False, num_devices=NC)

    xT_d = nc.dram_tensor("xT", [E, S], dt.bfloat16, kind="ExternalInput")
    wqkT_d = nc.dram_tensor("wqkT", [E, 512], dt.bfloat16, kind="ExternalInput")
    wvT_d = nc.dram_tensor("wvT", [E, 256], dt.bfloat16, kind="ExternalInput")
    bq_d = nc.dram_tensor("bq", [256], dt.float32, kind="ExternalInput")
    wo_d = nc.dram_tensor("wo", [256, E], dt.bfloat16, kind="ExternalInput")
    mask_d = nc.dram_tensor("trimask", [128, 128], dt.float32, kind="ExternalInput")
    y_d = nc.dram_tensor("y", [S, E], dt.bfloat16, kind="ExternalOutput")

    dbg = {}
    if DEBUG_OUTS:
        dbg["qkt"] = nc.dram_tensor("dbg_qkt", [4, 128, S], dt.bfloat16, kind="ExternalOutput")
        dbg["pair"] = nc.dram_tensor("dbg_pair", [2, 128, S], dt.bfloat16, kind="ExternalOutput")
        dbg["den"] = nc.dram_tensor("dbg_den", [NCH, 2048], dt.float32, kind="ExternalOutput")
        dbg["recip"] = nc.dram_tensor("dbg_recip", [NCH, 2048], dt.bfloat16, kind="ExternalOutput")
        dbg["bcs"] = nc.dram_tensor("dbg_bcs", [NCH, 128, 1024], dt.bfloat16, kind="ExternalOutput")
        dbg["vones"] = nc.dram_tensor("dbg_vones", [NT, 128, 260], dt.bfloat16, kind="ExternalOutput")

    with TileKernel(nc) as tk:
        tk.dbg = dbg
        tk.build(xT_d, wqkT_d, wvT_d, bq_d, wo_d, mask_d, y_d)
    nc.compile()
    return nc


class TileKernel:
    def __init__(self, nc):
        self.nc = nc
        self.dbg = {}
        self.ctx = ExitStack()
        self.tc_cm = tile.TileContext(nc)

    def __enter__(self):
        self.tc = self.tc_cm.__enter__()
        return self

    def __exit__(self, *a):
        self.ctx.close()
        return self.tc_cm.__exit__(*a)

    def build(self, xT_d, wqkT_d, wvT_d, bq_d, wo_d, mask_d, y_d):
        nc, tc, ctx = self.nc, self.tc, self.ctx
        pool = lambda name, bufs, **kw: ctx.enter_context(
            tc.tile_pool(name=name, bufs=bufs, **kw)
        )

        const_p = pool("const", 1)
        xs_p = pool("xs", 1)
        qkt_p = pool("qkt", 1)
        vones_p = pool("vones", 1)
        attn_p = pool("attn", PEND + 2)
        pair_p = pool("pair", 1)
        small_p = pool("small", 2)
        y_p = pool("y", 3)
        ps_p = pool("ps", 3, space="PSUM")     # 3 x [128,1024] = 6 banks
        po_p = pool("po", 1, space="PSUM")     # [65, 2, 512] shared = 2 banks

        # ---- small consts first (fast DMAs) ----
        mask_sb = const_p.tile([128, 128], dt.float32, tag="mask")
        nc.sync.dma_start(mask_sb[:], mask_d[:])
        bq_sb = const_p.tile([128, 2], dt.float32, tag="bq")
        nc.sync.dma_start(bq_sb[:], bq_d[:].rearrange("(f p) -> p f", p=128))

        # ---- weights + x, interleaved per-ke so chunk-0 QKV starts early ----
        wqk_sb = const_p.tile([128, KE, 512], dt.bfloat16, tag="wqk")
        xs = xs_p.tile([128, KE, S], dt.bfloat16, tag="xs", name="xs")
        for ke in range(KE):
            nc.sync.dma_start(
                wqk_sb[:, ke, :],
                wqkT_d[128 * ke : 128 * (ke + 1), :],
            )
            nc.sync.dma_start(
                xs[:, ke, 0:512],
                xT_d[128 * ke : 128 * (ke + 1), 0:512],
            )
        wv_sb = const_p.tile([128, KE, 256], dt.bfloat16, tag="wv")
        nc.sync.dma_start(
            wv_sb[:],
            wvT_d[:].rearrange("(ke p) f -> p ke f", p=128),
        )
        for c in range(1, NCH):
            if c == 2:
                wo_sb = const_p.tile([128, 2, E], dt.bfloat16, tag="wo")
                nc.sync.dma_start(
                    wo_sb[:],
                    wo_d[:].rearrange("(kt p) f -> p kt f", p=128),
                )
            cs = slice(512 * c, 512 * (c + 1))
            nc.sync.dma_start(
                xs[:, :, cs],
                xT_d[:, cs].rearrange("(ke p) f -> p ke f", p=128),
            )

        # ---- persistent activations ----
        # qkt tiles: 0: q heads 0,1 | 1: q heads 2,3 | 2: k heads 0,1 | 3: k heads 2,3
        qkt = [qkt_p.tile([128, S], dt.bfloat16, tag=f"qkt{f}", name=f"qkt{f}")
               for f in range(4)]
        # vones[t]: per head [v(64) | 1] -> [128, 4, 65]
        vones = [vones_p.tile([128, 4, 65], dt.bfloat16, tag=f"v{t}", name=f"v{t}")
                 for t in range(NT)]
        for t in range(NT):
            nc.vector.memset(vones[t][:, :, 64:65], 1.0)
        # pairt[kt]: normalized attn output, [2 heads x 64 dims, S]
        pairt = [pair_p.tile([128, S], dt.bfloat16, tag=f"pair{hp}", name=f"pair{hp}")
                 for hp in range(2)]

        env = dict(
            xs=xs, wqk_sb=wqk_sb, wv_sb=wv_sb, bq_sb=bq_sb, wo_sb=wo_sb,
            mask_sb=mask_sb, qkt=qkt, vones=vones, pairt=pairt,
            xs_p=xs_p, ps_p=ps_p, po_p=po_p, attn_p=attn_p, small_p=small_p,
            y_p=y_p, y_d=y_d, po={}, recip={},
        )

        # startup: chunk-0 qkv emitted directly
        for u in self.qkv_units(0, env):
            u()
        for c in range(NCH):
            fillers = []
            if c == 1:
                fillers += self.r2_units(0, env) + self.qkv_units(2, env)
            elif c == 2:
                fillers += self.r2_units(1, env) + self.oproj_units(0, env)
                fillers += self.qkv_units(3, env)
            elif c == 3:
                fillers += self.r2_units(2, env)
                fillers += self.oproj_units(1, env) + self.oproj_units(2, env)
            elif c == 0:
                fillers += self.qkv_units(1, env)
            self.attention_chunk(c, env, fillers)
        if self.dbg:
            for f in range(4):
                nc.sync.dma_start(self.dbg["qkt"][f], qkt[f][:])
            for hp in range(2):
                nc.sync.dma_start(self.dbg["pair"][hp], pairt[hp][:])
            for t in range(NT):
                nc.sync.dma_start(
                    self.dbg["vones"][t],
                    vones[t][:].rearrange("p g d -> p (g d)"),
                )

    # ------------------------------------------------------------------
    def qkv_units(self, c, env):
        nc = self.nc
        cs = slice(512 * c, 512 * (c + 1))
        xs, wqk_sb, wv_sb = env["xs"], env["wqk_sb"], env["wv_sb"]
        bq_sb, qkt, vones = env["bq_sb"], env["qkt"], env["vones"]
        ps_p = env["ps_p"]
        units = []

        def qk_unit(f):
            pq = ps_p.tile([128, 1024], dt.float32, tag="ps", name="pq")
            for ke in range(KE):
                nc.tensor.matmul(
                    pq[:, 0:512],
                    wqk_sb[:, ke, 128 * f : 128 * (f + 1)],
                    xs[:, ke, cs],
                    start=(ke == 0), stop=(ke == KE - 1),
                )
            if f < 2:
                nc.vector.tensor_scalar_add(
                    qkt[f][:, cs], pq[:, 0:512], bq_sb[:, f : f + 1])
            else:
                nc.vector.tensor_copy(qkt[f][:, cs], pq[:, 0:512])

        def v_unit(t4):
            t = 4 * c + t4
            pv = ps_p.tile([128, 1024], dt.float32, tag="ps", name="pv")
            for ke in range(KE):
                nc.tensor.matmul(
                    pv[:, 0:256],
                    xs[:, ke, 512 * c + 128 * t4 : 512 * c + 128 * (t4 + 1)],
                    wv_sb[:, ke, :],
                    start=(ke == 0), stop=(ke == KE - 1),
                )
            nc.vector.tensor_copy(
                vones[t][:, :, 0:64],
                pv[:, 0:256].rearrange("p (g d) -> p g d", d=64),
            )

        for f in range(4):
            units.append(lambda f=f: qk_unit(f))
        for t4 in range(4):
            units.append(lambda t4=t4: v_unit(t4))
        return units

    # ------------------------------------------------------------------
    def r2_units(self, c, env):
        """Broadcast reciprocal denominators and normalize pairt in place."""
        nc = self.nc
        pairt = env["pairt"]
        cs = slice(512 * c, 512 * (c + 1))
        bcs = {}

        def bc_unit(hp):
            recrow = env["recip"][(c, hp)]
            sb = env["small_p"].tile([128, 512], dt.float32, tag=f"bcs{hp}",
                                     name="bcs")
            for h in range(2):
                nc.sync.dma_start(
                    sb[64 * h : 64 * (h + 1), :],
                    recrow[0:1, 512 * h : 512 * (h + 1)]
                    .rearrange("a (o n) -> a o n", o=1)
                    .to_broadcast((1, 64, 512)),
                )
            bcs[hp] = sb

        def mult_unit(hp):
            bc = bcs[hp]
            for h in range(2):
                sl = pairt[hp][64 * h : 64 * (h + 1), cs]
                eng = nc.vector if h == 0 else nc.gpsimd
                eng.tensor_mul(sl, sl, bc[64 * h : 64 * (h + 1), :])

        return [lambda: bc_unit(0), lambda: mult_unit(0),
                lambda: bc_unit(1), lambda: mult_unit(1)]

    # ------------------------------------------------------------------
    def oproj_units(self, c, env):
        nc = self.nc
        pairt, wo_sb, ps_p, y_p, y_d = (
            env["pairt"], env["wo_sb"], env["ps_p"], env["y_p"], env["y_d"])
        units = []

        def unit(t4):
            t = 4 * c + t4
            ysb = y_p.tile([128, E], dt.bfloat16, tag="y", name="ysb")
            py = ps_p.tile([128, 1024], dt.float32, tag="ps", name="py")
            for o in range(2):
                for kt in range(2):
                    nc.tensor.matmul(
                        py[:, 512 * o : 512 * (o + 1)],
                        pairt[kt][:, 128 * t : 128 * (t + 1)],
                        wo_sb[:, kt, 512 * o : 512 * (o + 1)],
                        start=(kt == 0), stop=(kt == 1),
                    )
            nc.vector.tensor_copy(ysb[:], py[:])
            eng = nc.sync if t % 2 == 0 else nc.scalar
            eng.dma_start(y_d[128 * t : 128 * (t + 1), :], ysb[:])

        for t4 in range(4):
            units.append(lambda t4=t4: unit(t4))
        return units

    # ------------------------------------------------------------------
    def attention_chunk(self, c, env, fillers):
        """Attention for both head pairs of chunk c, weaving filler units
        (prev-chunk rollout/out-proj, next-chunk qkv) into the PE stream."""
        nc = self.nc
        qkt, vones, mask_sb = env["qkt"], env["vones"], env["mask_sb"]
        ps_p, po_p, attn_p = env["ps_p"], env["po_p"], env["attn_p"]
        nj = 4 * c + 4

        nfill = len(fillers)
        iters = 2 * nj
        emitted = 0

        def emit_pv(hp, j, off, at):
            for h in range(2):
                i = 2 * hp + h
                nc.tensor.matmul(
                    po_t[hp][:, h, off:512],
                    vones[j][:, i, :],
                    at[:, 512 * h + off : 512 * (h + 1)],
                    start=(j == 0), stop=(j == nj - 1),
                    skip_group_check=True,
                )

        it = 0
        po_t = [None, None]
        for hp in range(2):
            po_t[hp] = po_p.tile([65, 2, 512], dt.float32, tag="po", name="po")
            pending = []
            for j in range(nj):
                at = attn_p.tile([128, 1024], dt.bfloat16, tag="attn", name="at")
                ps = ps_p.tile([128, 1024], dt.float32, tag="ps", name="ps")
                m = j - 4 * c
                off = 128 * m if m >= 1 else 0
                for h in range(2):
                    r0 = 64 * h
                    nc.tensor.matmul(
                        ps[:, 512 * h + off : 512 * (h + 1)],
                        qkt[2 + hp][r0 : r0 + 64, 128 * j : 128 * (j + 1)],
                        qkt[hp][r0 : r0 + 64, 512 * c + off : 512 * (c + 1)],
                        start=True, stop=True,
                    )
                    if m >= 0:
                        lo = 512 * h + 128 * m
                        nc.vector.tensor_add(
                            ps[:, lo : lo + 128], ps[:, lo : lo + 128],
                            mask_sb[:],
                        )
                if off == 0:
                    nc.scalar.activation(at[:], ps[:], AF.Exp)
                else:
                    for h in range(2):
                        nc.scalar.activation(
                            at[:, 512 * h + off : 512 * (h + 1)],
                            ps[:, 512 * h + off : 512 * (h + 1)], AF.Exp)
                pending.append((j, off, at))
                if len(pending) > PEND:
                    emit_pv(hp, *pending.pop(0))
                it += 1
                while emitted < nfill and emitted * iters < it * nfill:
                    fillers[emitted]()
                    emitted += 1
            if c == NCH - 1 and hp == 1:
                for p in pending:
                    emit_pv(hp, *p)
                    self.rollout_qtile(c, p[0] - 4 * c, po_t[1], env)
            else:
                for p in pending:
                    emit_pv(hp, *p)
                self.rollout_hp(c, hp, po_t[hp], env)
                if c == NCH - 1:
                    # hp0 normalize for the last chunk, resolved during hp1
                    r2 = self.r2_units(c, env)
                    r2[0](); r2[1]()
        while emitted < nfill:
            fillers[emitted]()
            emitted += 1

    def rollout_hp(self, c, hp, po, env):
        """Copy unnormalized attn output to pairt (freeing po) and compute
        1/denominator: ACT Copy (no table switch) -> DMA reshape to [128,8]
        -> wide DVE reciprocal -> DMA back to a row for broadcasting."""
        nc = self.nc
        pairt = env["pairt"]
        cs = slice(512 * c, 512 * (c + 1))
        for h in range(2):
            nc.vector.tensor_copy(
                pairt[hp][64 * h : 64 * (h + 1), cs], po[0:64, h, :])
        denrow = env["small_p"].tile([1, 1024], dt.float32, tag=f"recf{hp}",
                                     name="denrow")
        nc.scalar.activation(denrow[0:1, :], po[64:65, :, :], AF.Copy)
        dencol = env["small_p"].tile([128, 8], dt.float32, tag=f"denc{hp}",
                                     name="dencol")
        nc.sync.dma_start(dencol[:], denrow[0:1, :])
        reccol = env["small_p"].tile([128, 8], dt.float32, tag=f"recc{hp}",
                                     name="reccol")
        nc.vector.reciprocal(reccol[:], dencol[:])
        recrow = env["small_p"].tile([1, 1024], dt.float32, tag=f"recb{hp}",
                                     name="recrow")
        nc.sync.dma_start(recrow[0:1, :], reccol[:])
        env["recip"][(c, hp)] = recrow

    def rollout_qtile(self, c, m, po, env):
        """Last-chunk hp1: normalize one query tile and immediately run its
        output projection + y store, pipelined against the remaining PVs."""
        nc = self.nc
        pairt, ps_p, y_p, y_d = env["pairt"], env["ps_p"], env["y_p"], env["y_d"]
        wo_sb = env["wo_sb"]
        t = 4 * c + m
        qs = slice(128 * t, 128 * (t + 1))
        ms = slice(128 * m, 128 * (m + 1))
        for h in range(2):
            nc.vector.tensor_copy(
                pairt[1][64 * h : 64 * (h + 1), qs], po[0:64, h, ms])
        sp = env["small_p"]
        denrow = sp.tile([1, 256], dt.float32, tag=f"qden{m}", name="denrow")
        nc.scalar.activation(denrow[0:1, :], po[64:65, :, ms], AF.Copy)
        dencol = sp.tile([32, 8], dt.float32, tag=f"qdenc{m}", name="dencol")
        nc.sync.dma_start(dencol[:], denrow[0:1, :])
        reccol = sp.tile([32, 8], dt.float32, tag=f"qrecc{m}", name="reccol")
        nc.vector.reciprocal(reccol[:], dencol[:])
        recrow = sp.tile([1, 256], dt.float32, tag=f"qrecb{m}", name="recrow")
        nc.sync.dma_start(recrow[0:1, :], reccol[:])
        bcsq = sp.tile([128, 128], dt.float32, tag=f"qbcs{m}", name="bcsq")
        for h in range(2):
            nc.sync.dma_start(
                bcsq[64 * h : 64 * (h + 1), :],
                recrow[0:1, 128 * h : 128 * (h + 1)]
                .rearrange("a (o n) -> a o n", o=1)
                .to_broadcast((1, 64, 128)),
            )
        for h in range(2):
            sl = pairt[1][64 * h : 64 * (h + 1), qs]
            eng = nc.vector if h == 0 else nc.gpsimd
            eng.tensor_mul(sl, sl, bcsq[64 * h : 64 * (h + 1), :])
        ysb = y_p.tile([128, E], dt.bfloat16, tag="y", name="ysb")
        py = ps_p.tile([128, 1024], dt.float32, tag="ps", name="py")
        for o in range(2):
            for kt in range(2):
                nc.tensor.matmul(
                    py[:, 512 * o : 512 * (o + 1)],
                    pairt[kt][:, qs],
                    wo_sb[:, kt, 512 * o : 512 * (o + 1)],
                    start=(kt == 0), stop=(kt == 1),
                )
        nc.vector.tensor_copy(ysb[:], py[:])
        nc.scalar.dma_start(y_d[qs, :], ysb[:])
        if self.dbg:
            den_sb = env["small_p"].tile([1, 2048], dt.float32, tag="dens", name="dens")
            nc.vector.tensor_copy(den_sb[0:1, :], po[64:65, :, :])
            nc.sync.dma_start(self.dbg["den"][c], den_sb[0:1, :].rearrange("a n -> (a n)"))
            nc.sync.dma_start(self.dbg["recip"][c], recip_bf[0:1, :].rearrange("a n -> (a n)"))


# ----------------------------------------------------------------------
_PROGRAM = None


def _get_program():
    global _PROGRAM
    if _PROGRAM is None:
        _PROGRAM = _build_program()
    return _PROGRAM


def _make_in_maps(inputs, W_in, b_in, W_out, b_out):
    in_maps = []
    bf16 = ml_dtypes.bfloat16
    scale = 1.0 / np.sqrt(np.float32(HD))
    kr = np.arange(128)[:, None]
    qc = np.arange(128)[None, :]
    trimask = np.where(qc >= kr, 0.0, -1e30).astype(np.float32)
    for core in range(NC):
        b, g = divmod(core, 4)
        r = slice(256 * g, 256 * (g + 1))
        wq = W_in[0:E][r] * scale
        wk = W_in[E : 2 * E][r]
        wv = W_in[2 * E : 3 * E][r]
        xT = np.ascontiguousarray(inputs[b].T).astype(bf16)
        wqkT = np.ascontiguousarray(np.concatenate([wq, wk], axis=0).T).astype(bf16)
        wvT = np.ascontiguousarray(wv.T).astype(bf16)
        bq = (b_in[0:E][r] * scale).astype(np.float32)
        wo = np.ascontiguousarray(W_out[:, r].T).astype(bf16)
        in_maps.append(
            {
                "xT": xT,
                "wqkT": wqkT,
                "wvT": wvT,
                "bq": bq,
                "wo": wo,
                "trimask": trimask,
            }
        )
    return in_maps


def run_spmd(inputs, W_in, b_in, W_out, b_out, trace=False, **kw):
    nc = _get_program()
    in_maps = _make_in_maps(inputs, W_in, b_in, W_out, b_out)
    bkr = run_bass_kernel_spmd(nc, in_maps, list(range(NC)), trace=trace, **kw)
    parts = [bkr.results[i]["y"].astype(np.float32) for i in range(NC)]
    out = np.stack(
        [
            parts[0] + parts[1] + parts[2] + parts[3],
            parts[4] + parts[5] + parts[6] + parts[7],
        ]
    )
    yb = W_out.astype(np.float32) @ b_in[2 * E : 3 * E].astype(np.float32)
    out = out + (yb + b_out)[None, None, :]
    return out.astype(np.float32), bkr


def kernel(inputs, W_in, b_in, W_out, b_out):
    out, _ = run_spmd(
        np.asarray(inputs, dtype=np.float32),
        np.asarray(W_in, dtype=np.float32),
        np.asarray(b_in, dtype=np.float32),
        np.asarray(W_out, dtype=np.float32),
        np.asarray(b_out, dtype=np.float32),
    )
    return out
